# revision 13
# baseline (speedup 1.0000x reference)
"""Gemma-style transformer block (GQA + sliding-window attention + gated-GELU
MLP) on 8 Trainium2 NeuronCores.

Sharding (Megatron + sequence-parallel):
  - Attention: tensor-parallel over heads. Core c owns q heads {2c, 2c+1} and
    kv head c. Each core computes full-sequence attention for its heads plus
    its partial o-projection; a per-batch ReduceScatter (over tokens) completes
    the sum, leaving core c with tokens [128c, 128c+128) of each batch.
  - Norms + residuals run on the token shard (sequence-parallel).
  - MLP: tensor-parallel over the hidden dim (core c owns HID slice
    [1024c, 1024c+1024)). An AllGather of the (transposed) h2 shard feeds the
    gate/up matmuls; a second per-batch ReduceScatter completes down-proj.
  - Host concatenates the 8 per-core [256, D] output shards.

Matmuls run as float32r (full-rate fp32 variant, ~1.6e-4 component error);
everything else is fp32.
"""
import sys

sys.path.insert(0, "/opt/trn_rl_repo")

import numpy as np

import concourse.bass as bass
import concourse.mybir as mybir
import concourse.tile as tile
from concourse import bacc

F32 = mybir.dt.float32
F32R = mybir.dt.float32r
AF = mybir.ActivationFunctionType
OP = mybir.AluOpType

B, T, D = 2, 1024, 2048
NQ, KV, H, HID = 16, 8, 128, 8192
WINDOW, CAP = 512, 50.0
KMASK = -2.3819763e38
EPS = 1e-6
ROPE_BASE = 10000.0
NCORES = 8
DT = D // 128          # 16 contraction tiles over D
TB = T // 128          # 8 token blocks per batch
RG = [list(range(NCORES))]


def _midx(qb, kb):
    """Canonical additive-mask tile index for (query block, key block)."""
    if kb > qb:
        return 0       # future block: fully masked
    if kb == qb:
        return 1       # causal lower-tri (incl diag)
    if kb == qb - 4:
        return 3       # window tail: strict upper-tri allowed
    if kb < qb - 4:
        return 0       # fully outside window
    return 2           # fully inside window: no mask


def _rms(nc, pools, in_ap, out_ap, mul_bc, width, sq_tile):
    """out = in * rsqrt(mean(in^2)+EPS) * mul   (mul broadcast tile)."""
    ss = pools["small"].tile([128, 1], F32)
    nc.scalar.activation(sq_tile, in_ap, AF.Square, accum_out=ss[:])
    rs = pools["small"].tile([128, 1], F32)
    nc.scalar.activation(rs[:], ss[:], AF.Sqrt, scale=1.0 / width,
                         bias=pools["eps"][:])
    nc.vector.reciprocal(rs[:], rs[:])
    nc.vector.scalar_tensor_tensor(out=out_ap, in0=in_ap, scalar=rs[:],
                                   in1=mul_bc, op0=OP.mult, op1=OP.mult)


def _bcast_row(nc, dst, src_ap):
    """DMA a [W] dram vector broadcast to a [P, W] sbuf tile."""
    nc.sync.dma_start(dst, bass.AP(
        tensor=src_ap.tensor, offset=src_ap.offset,
        ap=[[0, dst.shape[0]], *src_ap.ap]))


def _build_program():
    nc = bacc.Bacc("TRN2", target_bir_lowering=False, debug=False,
                   enable_asserts=True, num_devices=NCORES)

    def din(name, shape, dt=F32):
        return nc.dram_tensor(name, shape, dt, kind="ExternalInput").ap()

    x = din("x", [B * T, D])
    xsh = din("xsh", [2 * 128, D])
    wqkv = din("wqkv", [D, 512], F32R)          # [D, 2H q | H k | H v]
    ow = din("ow", [256, D], F32R)              # [2*H rows, D]
    gw = din("gw", [D, 1024], F32R)
    uw = din("uw", [D, 1024], F32R)
    dw = din("dw", [1024, D], F32R)
    cosb = din("cosb", [B * T, 64])
    sinb = din("sinb", [B * T, 64])
    maskb = din("maskb", [4, 128, 128])
    premul = din("premul", [D])
    postattnmul = din("postattnmul", [D])
    preffwmul = din("preffwmul", [D])
    postffwmul = din("postffwmul", [D])
    qmul = din("qmul", [H])
    kmul = din("kmul", [H])
    iden = din("iden", [128, 128], F32R)

    out = nc.dram_tensor("out", [2 * 128, D], F32, kind="ExternalOutput").ap()

    with tile.TileContext(nc) as tc:
        _body(nc, tc, x=x, xsh=xsh, wqkv=wqkv, ow=ow, gw=gw, uw=uw, dw=dw,
              cosb=cosb, sinb=sinb, maskb=maskb, premul=premul,
              postattnmul=postattnmul, preffwmul=preffwmul,
              postffwmul=postffwmul, qmul=qmul, kmul=kmul, iden=iden, out=out)
    nc.compile()
    return nc


def _body(nc, tc, *, x, xsh, wqkv, ow, gw, uw, dw, cosb, sinb, maskb, premul,
          postattnmul, preffwmul, postffwmul, qmul, kmul, iden, out):
    from contextlib import ExitStack

    est = ExitStack()
    with est:
        # ----- long-lived pools -----
        consts = est.enter_context(tc.tile_pool(name="consts", bufs=1))
        bcast = est.enter_context(tc.tile_pool(name="bcast", bufs=2))
        small = est.enter_context(tc.tile_pool(name="small", bufs=8))
        obp = est.enter_context(tc.tile_pool(name="obp", bufs=2))
        bwork = est.enter_context(tc.tile_pool(name="bwork", bufs=3))
        dram = est.enter_context(tc.tile_pool(name="dram", bufs=1,
                                              space="DRAM"))
        psA = est.enter_context(tc.tile_pool(name="psA", bufs=3, space="PSUM"))
        psT = est.enter_context(tc.tile_pool(name="psT", bufs=2, space="PSUM"))

        iden_sb = consts.tile([128, 128], F32R)
        nc.sync.dma_start(iden_sb[:], iden[:])
        qmul_bc = consts.tile([128, H], F32)
        _bcast_row(nc, qmul_bc[:], qmul)
        kmul_bc = consts.tile([128, H], F32)
        _bcast_row(nc, kmul_bc[:], kmul)
        eps_t = consts.tile([128, 1], F32)
        nc.vector.memset(eps_t[:], EPS)
        pools = {"small": small, "eps": eps_t}

        premul_bc = bcast.tile([128, D], F32, tag="bc")
        _bcast_row(nc, premul_bc[:], premul)

        # DRAM intermediates
        attn_out_d = dram.tile([B * 128, D], F32)
        o_part = dram.tile([B * T, D], F32)
        rs1 = dram.tile([B * 128, D], F32)
        ag_in = dram.tile([B * D, 128], F32R)
        ag_out = dram.tile([B * NCORES * D, 128], F32R)
        mlp_part = dram.tile([B * T, D], F32)
        rs2 = dram.tile([B * 128, D], F32)

        # =================== ATTENTION (TP over heads) ===================
        with ExitStack() as attn_scope:
            ap_ = {}
            for nm, args in [
                ("csin", dict(bufs=2)), ("maskp", dict(bufs=1)),
                ("wqkvp", dict(bufs=1)), ("owp", dict(bufs=1)),
                ("hT", dict(bufs=1)), ("hp", dict(bufs=1)),
                ("sqs", dict(bufs=1)), ("nrm", dict(bufs=2)),
                ("ro", dict(bufs=2)), ("t64", dict(bufs=2)),
                ("qT", dict(bufs=1)), ("kT", dict(bufs=1)),
                ("vp", dict(bufs=1)), ("t1", dict(bufs=1)),
                ("exp", dict(bufs=1)), ("prob", dict(bufs=1)),
                ("pT", dict(bufs=1)), ("encp", dict(bufs=2)),
                ("bh2", dict(bufs=1)),
            ]:
                ap_[nm] = attn_scope.enter_context(
                    tc.tile_pool(name=nm, **args))
            psL = attn_scope.enter_context(
                tc.tile_pool(name="psL", bufs=1, space="PSUM"))

            maskb_sb = ap_["maskp"].tile([128, 4, 128], F32)
            nc.sync.dma_start(maskb_sb[:], maskb.rearrange("m p k -> p m k"))
            wqkv_sb = ap_["wqkvp"].tile([128, DT, 512], F32R)
            nc.sync.dma_start(wqkv_sb[:],
                              wqkv.rearrange("(dt p) c -> p dt c", p=128))
            ow_sb = ap_["owp"].tile([128, 2, 4, 512], F32R)
            for hh in range(2):
                for ch in range(4):
                    nc.sync.dma_start(
                        ow_sb[:, hh, ch, :],
                        ow[hh * 128:(hh + 1) * 128, ch * 512:(ch + 1) * 512])

            for b in range(B):
                cos_t = ap_["csin"].tile([128, TB, 64], F32, tag="cs")
                nc.sync.dma_start(
                    cos_t[:], cosb[b * T:(b + 1) * T, :].rearrange(
                        "(tb p) h -> p tb h", p=128))
                sin_t = ap_["csin"].tile([128, TB, 64], F32, tag="cs")
                nc.sync.dma_start(
                    sin_t[:], sinb[b * T:(b + 1) * T, :].rearrange(
                        "(tb p) h -> p tb h", p=128))

                qT = ap_["qT"].tile([128, 2, T], F32R)
                kT = ap_["kT"].tile([128, T], F32R)
                v_sb = ap_["vp"].tile([128, TB, 128], F32R)

                for hf in range(4):       # quarter-batch over tokens
                    hT = ap_["hT"].tile([128, DT, 256], F32R)
                    # ---- h = rms(x)*premul, transposed into hT ----
                    for t4 in range(2):
                        tb = hf * 2 + t4
                        xt = bwork.tile([128, D], F32, tag="bw")
                        nc.sync.dma_start(
                            xt[:], x[b * T + tb * 128: b * T + (tb + 1) * 128, :])
                        ht = ap_["hp"].tile([128, D], F32R)
                        sq = bwork.tile([128, D], F32, tag="bw")
                        _rms(nc, pools, xt[:], ht[:], premul_bc[:], D, sq[:])
                        for dt in range(DT):
                            pt = psT.tile([128, 128], F32R)
                            nc.tensor.transpose(
                                pt[:], ht[:, dt * 128:(dt + 1) * 128],
                                iden_sb[:])
                            nc.vector.tensor_copy(
                                hT[:, dt, t4 * 128:(t4 + 1) * 128], pt[:])
                    # ---- qkv projections + qk-norm + rope ----
                    for t4 in range(2):
                        tb = hf * 2 + t4
                        pq = psA.tile([128, 512], F32, tag="mm")
                        for dt in range(DT):
                            nc.tensor.matmul(
                                pq[:], hT[:, dt, t4 * 128:(t4 + 1) * 128],
                                wqkv_sb[:, dt, :],
                                start=(dt == 0), stop=(dt == DT - 1))
                        for hd in range(3):    # q0, q1, k
                            sl = pq[:, hd * 128:(hd + 1) * 128]
                            nrm = ap_["nrm"].tile([128, 128], F32)
                            sqs = ap_["sqs"].tile([128, 128], F32)
                            _rms(nc, pools, sl, nrm[:],
                                 (qmul_bc if hd < 2 else kmul_bc)[:], H,
                                 sqs[:])
                            ro = ap_["ro"].tile([128, 128], F32R)
                            ct, st = cos_t[:, tb, :], sin_t[:, tb, :]
                            t1 = ap_["t64"].tile([128, 64], F32)
                            t2 = ap_["t64"].tile([128, 64], F32)
                            nc.vector.tensor_mul(t1[:], nrm[:, 0:64], ct)
                            nc.vector.tensor_mul(t2[:], nrm[:, 64:128], st)
                            nc.vector.tensor_sub(ro[:, 0:64], t1[:], t2[:])
                            t3 = ap_["t64"].tile([128, 64], F32)
                            t4_ = ap_["t64"].tile([128, 64], F32)
                            nc.vector.tensor_mul(t3[:], nrm[:, 64:128], ct)
                            nc.vector.tensor_mul(t4_[:], nrm[:, 0:64], st)
                            nc.vector.tensor_add(ro[:, 64:128], t3[:], t4_[:])
                            pt = psT.tile([128, 128], F32R)
                            nc.tensor.transpose(pt[:], ro[:], iden_sb[:])
                            dst = (qT[:, hd, tb * 128:(tb + 1) * 128]
                                   if hd < 2 else
                                   kT[:, tb * 128:(tb + 1) * 128])
                            nc.vector.tensor_copy(dst, pt[:])
                        nc.vector.tensor_copy(v_sb[:, tb, :], pq[:, 384:512])

                # ---- banded attention, query-block pairs ----
                for p in range(4):
                    ukb0 = max(0, 2 * p - 4)
                    nu = 2 * p + 2 - ukb0
                    keys = nu * 128
                    enc = ap_["encp"].tile([128, 2, 256], F32R)
                    for h in range(2):
                        probsT = ap_["pT"].tile([128, 6, 256], F32R)
                        for qi in range(2):
                            qb = 2 * p + qi
                            psl = psL.tile([128, 768], F32)
                            c0 = 0
                            while c0 < keys:
                                c1 = min(c0 + 512, keys)
                                nc.tensor.matmul(
                                    psl[:, c0:c1],
                                    qT[:, h, qb * 128:(qb + 1) * 128],
                                    kT[:, ukb0 * 128 + c0: ukb0 * 128 + c1],
                                    start=True, stop=True)
                                c0 = c1
                            t1 = ap_["t1"].tile([128, 768], F32)
                            nc.scalar.activation(t1[:, 0:keys], psl[:, 0:keys],
                                                 AF.Tanh, scale=1.0 / CAP)
                            for j in range(nu):
                                mi = _midx(qb, ukb0 + j)
                                nc.vector.scalar_tensor_tensor(
                                    out=t1[:, j * 128:(j + 1) * 128],
                                    in0=t1[:, j * 128:(j + 1) * 128],
                                    scalar=CAP, in1=maskb_sb[:, mi, :],
                                    op0=OP.mult, op1=OP.add)
                            ex = ap_["exp"].tile([128, 768], F32)
                            den = small.tile([128, 1], F32)
                            nc.scalar.activation(ex[:, 0:keys], t1[:, 0:keys],
                                                 AF.Exp, accum_out=den[:])
                            nc.vector.reciprocal(den[:], den[:])
                            prob = ap_["prob"].tile([128, 768], F32R)
                            nc.vector.tensor_scalar_mul(
                                prob[:, 0:keys], ex[:, 0:keys], den[:])
                            for j in range(nu):
                                pt = psT.tile([128, 128], F32R)
                                nc.tensor.transpose(
                                    pt[:], prob[:, j * 128:(j + 1) * 128],
                                    iden_sb[:])
                                nc.vector.tensor_copy(
                                    probsT[:, j, qi * 128:(qi + 1) * 128],
                                    pt[:])
                        pav = psA.tile([128, 256], F32, tag="mm")
                        for j in range(nu):
                            nc.tensor.matmul(
                                pav[:], v_sb[:, ukb0 + j, :], probsT[:, j, :],
                                start=(j == 0), stop=(j == nu - 1))
                        nc.vector.tensor_copy(enc[:, h, :], pav[:])
                    # o-projection (partial over this core's 2 heads)
                    for qi in range(2):
                        qb = 2 * p + qi
                        for ch in range(4):
                            po = psA.tile([128, 512], F32, tag="mm")
                            for h in range(2):
                                nc.tensor.matmul(
                                    po[:], enc[:, h, qi * 128:(qi + 1) * 128],
                                    ow_sb[:, h, ch, :],
                                    start=(h == 0), stop=(h == 1))
                            ob = obp.tile([128, 512], F32)
                            nc.vector.tensor_copy(ob[:], po[:])
                            nc.sync.dma_start(
                                o_part[b * T + qb * 128: b * T + (qb + 1) * 128,
                                       ch * 512:(ch + 1) * 512], ob[:])

                nc.gpsimd.collective_compute(
                    "ReduceScatter", OP.add, replica_groups=RG,
                    ins=[o_part[b * T:(b + 1) * T, :].opt()],
                    outs=[rs1[b * 128:(b + 1) * 128, :].opt()])

            # ---------- sequence-parallel norm chain + AllGather ----------
            postattn_bc = bcast.tile([128, D], F32, tag="bc")
            _bcast_row(nc, postattn_bc[:], postattnmul)
            preffw_bc = bcast.tile([128, D], F32, tag="bc")
            _bcast_row(nc, preffw_bc[:], preffwmul)
            for b in range(B):
                rst = bwork.tile([128, D], F32, tag="bw")
                nc.sync.dma_start(rst[:], rs1[b * 128:(b + 1) * 128, :])
                ao = bwork.tile([128, D], F32, tag="bw")
                _rms(nc, pools, rst[:], ao[:], postattn_bc[:], D, ao[:])
                xst = bwork.tile([128, D], F32, tag="bw")
                nc.sync.dma_start(xst[:], xsh[b * 128:(b + 1) * 128, :])
                nc.vector.tensor_add(ao[:], ao[:], xst[:])
                nc.sync.dma_start(attn_out_d[b * 128:(b + 1) * 128, :], ao[:])
                h2 = ap_["bh2"].tile([128, D], F32R)
                sq2 = bwork.tile([128, D], F32, tag="bw")
                _rms(nc, pools, ao[:], h2[:], preffw_bc[:], D, sq2[:])
                h2T = ap_["bh2"].tile([128, DT, 128], F32R, tag="h2T")
                for dt in range(DT):
                    pt = psT.tile([128, 128], F32R)
                    nc.tensor.transpose(pt[:], h2[:, dt * 128:(dt + 1) * 128],
                                        iden_sb[:])
                    nc.vector.tensor_copy(h2T[:, dt, :], pt[:])
                nc.sync.dma_start(
                    ag_in[b * D:(b + 1) * D, :].rearrange(
                        "(dt p) c -> p dt c", p=128), h2T[:])
                nc.gpsimd.collective_compute(
                    "AllGather", OP.bypass, replica_groups=RG,
                    ins=[ag_in[b * D:(b + 1) * D, :].opt()],
                    outs=[ag_out[b * NCORES * D:(b + 1) * NCORES * D, :].opt()])

        # =================== MLP (TP over hidden dim) ===================
        with ExitStack() as mlp_scope:
            mp_ = {}
            for nm, args in [
                ("h2Tp", dict(bufs=1)), ("wst", dict(bufs=2)),
                ("actp", dict(bufs=1)), ("gel", dict(bufs=2)),
                ("dwp", dict(bufs=2)),
            ]:
                mp_[nm] = mlp_scope.enter_context(tc.tile_pool(name=nm, **args))

            for b in range(B):
                h2Tf = mp_["h2Tp"].tile([128, DT, 1024], F32R)
                base = b * NCORES * D
                for r in range(NCORES):
                    nc.sync.dma_start(
                        h2Tf[:, :, r * 128:(r + 1) * 128],
                        ag_out[base + r * D: base + (r + 1) * D, :].rearrange(
                            "(dt p) c -> p dt c", p=128))
                actT = mp_["actp"].tile([128, 8, 1024], F32R)
                for hc in range(8):
                    gw_t = mp_["wst"].tile([128, DT, 128], F32R, tag="w")
                    nc.sync.dma_start(
                        gw_t[:], gw[:, hc * 128:(hc + 1) * 128].rearrange(
                            "(dt p) h -> p dt h", p=128))
                    uw_t = mp_["wst"].tile([128, DT, 128], F32R, tag="w")
                    nc.sync.dma_start(
                        uw_t[:], uw[:, hc * 128:(hc + 1) * 128].rearrange(
                            "(dt p) h -> p dt h", p=128))
                    for tch in range(2):
                        psg = psA.tile([128, 512], F32, tag="mm")
                        psu = psA.tile([128, 512], F32, tag="mm")
                        for dt in range(DT):
                            nc.tensor.matmul(
                                psg[:], gw_t[:, dt, :],
                                h2Tf[:, dt, tch * 512:(tch + 1) * 512],
                                start=(dt == 0), stop=(dt == DT - 1))
                        for dt in range(DT):
                            nc.tensor.matmul(
                                psu[:], uw_t[:, dt, :],
                                h2Tf[:, dt, tch * 512:(tch + 1) * 512],
                                start=(dt == 0), stop=(dt == DT - 1))
                        gel = mp_["gel"].tile([128, 512], F32)
                        nc.scalar.activation(gel[:], psg[:],
                                             AF.Gelu_apprx_tanh)
                        nc.vector.tensor_mul(
                            actT[:, hc, tch * 512:(tch + 1) * 512],
                            gel[:], psu[:])
                # down-projection, streamed per 512-wide D chunk
                for ch in range(4):
                    dw_t = mp_["dwp"].tile([128, 8, 512], F32R)
                    for hc in range(8):
                        nc.sync.dma_start(
                            dw_t[:, hc, :],
                            dw[hc * 128:(hc + 1) * 128,
                               ch * 512:(ch + 1) * 512])
                    for tb in range(TB):
                        psd = psA.tile([128, 512], F32, tag="mm")
                        for hc in range(8):
                            nc.tensor.matmul(
                                psd[:], actT[:, hc, tb * 128:(tb + 1) * 128],
                                dw_t[:, hc, :],
                                start=(hc == 0), stop=(hc == 7))
                        mb = obp.tile([128, 512], F32)
                        nc.vector.tensor_copy(mb[:], psd[:])
                        nc.sync.dma_start(
                            mlp_part[b * T + tb * 128: b * T + (tb + 1) * 128,
                                     ch * 512:(ch + 1) * 512], mb[:])
                nc.gpsimd.collective_compute(
                    "ReduceScatter", OP.add, replica_groups=RG,
                    ins=[mlp_part[b * T:(b + 1) * T, :].opt()],
                    outs=[rs2[b * 128:(b + 1) * 128, :].opt()])

        # =================== final norm + residual ===================
        postffw_bc = bcast.tile([128, D], F32, tag="bc")
        _bcast_row(nc, postffw_bc[:], postffwmul)
        for b in range(B):
            rst = bwork.tile([128, D], F32, tag="bw")
            nc.sync.dma_start(rst[:], rs2[b * 128:(b + 1) * 128, :])
            tmp = bwork.tile([128, D], F32, tag="bw")
            _rms(nc, pools, rst[:], tmp[:], postffw_bc[:], D, tmp[:])
            aol = bwork.tile([128, D], F32, tag="bw")
            nc.sync.dma_start(aol[:], attn_out_d[b * 128:(b + 1) * 128, :])
            nc.vector.tensor_add(tmp[:], tmp[:], aol[:])
            nc.sync.dma_start(out[b * 128:(b + 1) * 128, :], tmp[:])


# ---------------------------------------------------------------------------
# host side
# ---------------------------------------------------------------------------

_NC = None


def _get_nc():
    global _NC
    if _NC is None:
        _NC = _build_program()
    return _NC


def _host_prep(inputs):
    """Build the 8 per-core input maps from the full problem inputs."""
    x = np.ascontiguousarray(np.asarray(inputs["x"], dtype=np.float32))
    seg = np.asarray(inputs["segment_pos"], dtype=np.int32)
    am = np.asarray(inputs["attn_mask"])
    q_k = np.asarray(inputs["q_kernel"], dtype=np.float32)
    kv_k = np.asarray(inputs["kv_kernel"], dtype=np.float32)
    o_k = np.asarray(inputs["o_kernel"], dtype=np.float32)
    gate_w = np.asarray(inputs["gate_w"], dtype=np.float32)
    up_w = np.asarray(inputs["up_w"], dtype=np.float32)
    down_w = np.asarray(inputs["down_w"], dtype=np.float32)

    xf = x.reshape(B * T, D)
    premul = (1.0 + np.asarray(inputs["pre_attn_scale"], np.float32))
    postattn = (1.0 + np.asarray(inputs["post_attn_scale"], np.float32))
    preffw = (1.0 + np.asarray(inputs["pre_ffw_scale"], np.float32))
    postffw = (1.0 + np.asarray(inputs["post_ffw_scale"], np.float32))
    qmul = ((1.0 + np.asarray(inputs["q_norm_scale"], np.float32))
            * np.float32(H ** -0.5)).astype(np.float32)
    kmul = (1.0 + np.asarray(inputs["k_norm_scale"], np.float32))

    frac = (2.0 * np.arange(H // 2, dtype=np.float32) / H).astype(np.float32)
    ts = (ROPE_BASE ** frac).astype(np.float32)
    sinu = (seg[..., None].astype(np.float32) / ts).astype(np.float32)
    cosb = np.cos(sinu).reshape(B * T, 64).astype(np.float32)
    sinb = np.sin(sinu).reshape(B * T, 64).astype(np.float32)

    ti = np.arange(128)[:, None]
    si = np.arange(128)[None, :]
    maskb = np.stack([
        np.full((128, 128), KMASK, np.float32),
        np.where(ti >= si, 0.0, KMASK).astype(np.float32),
        np.zeros((128, 128), np.float32),
        np.where(ti < si, 0.0, KMASK).astype(np.float32),
    ]).astype(np.float32)

    # soft structural check of the actual mask against the canonical tiles
    tt = np.arange(T)
    sliding = (np.abs(tt[:, None] - tt[None, :]) <= WINDOW - 1)
    expected = am & sliding[None]
    ok = True
    for qb in range(min(2, TB)):
        for kb in range(qb + 1):
            blk = np.where(expected[0, qb * 128:(qb + 1) * 128,
                                    kb * 128:(kb + 1) * 128], 0.0, KMASK)
            if not np.array_equal(blk.astype(np.float32),
                                  maskb[_midx(qb, kb)]):
                ok = False
    if not ok:
        print("kernel.py WARNING: attn_mask does not match canonical "
              "causal+sliding structure; results may be wrong")

    iden = np.eye(128, dtype=np.float32)

    in_maps = []
    for c in range(NCORES):
        qw_c = q_k[2 * c:2 * c + 2].transpose(1, 0, 2).reshape(D, 256)
        kw_c = kv_k[0, c]
        vw_c = kv_k[1, c]
        wqkv_c = np.ascontiguousarray(
            np.concatenate([qw_c, kw_c, vw_c], axis=1), dtype=np.float32)
        ow_c = np.ascontiguousarray(
            o_k[2 * c:2 * c + 2].reshape(256, D), dtype=np.float32)
        gw_c = np.ascontiguousarray(gate_w[:, 1024 * c:1024 * (c + 1)])
        uw_c = np.ascontiguousarray(up_w[:, 1024 * c:1024 * (c + 1)])
        dw_c = np.ascontiguousarray(down_w[1024 * c:1024 * (c + 1), :])
        xsh_c = np.ascontiguousarray(np.concatenate(
            [xf[128 * c:128 * (c + 1)],
             xf[T + 128 * c: T + 128 * (c + 1)]], axis=0))
        in_maps.append({
            "x": xf, "xsh": xsh_c, "wqkv": wqkv_c, "ow": ow_c,
            "gw": gw_c, "uw": uw_c, "dw": dw_c,
            "cosb": cosb, "sinb": sinb, "maskb": maskb,
            "premul": premul, "postattnmul": postattn,
            "preffwmul": preffw, "postffwmul": postffw,
            "qmul": qmul, "kmul": kmul, "iden": iden,
        })
    return in_maps


def _assemble(results):
    out = np.empty((B, T, D), dtype=np.float32)
    for c in range(NCORES):
        r = results[c]["out"]
        out[0, 128 * c:128 * (c + 1)] = r[0:128]
        out[1, 128 * c:128 * (c + 1)] = r[128:256]
    return out


def kernel(**inputs) -> np.ndarray:
    from concourse import bass_utils
    nc = _get_nc()
    in_maps = _host_prep(inputs)
    r = bass_utils.run_bass_kernel_spmd(nc, in_maps,
                                        core_ids=list(range(NCORES)))
    return _assemble(r.results)


# revision 15
# speedup vs baseline: 1.2312x; 1.2312x over previous
"""Gemma-style transformer block (GQA + sliding-window attention + gated-GELU
MLP) on 8 Trainium2 NeuronCores.

Sharding (Megatron + sequence-parallel):
  - Attention: tensor-parallel over heads. Core c owns q heads {2c, 2c+1} and
    kv head c. Each core computes full-sequence attention for its heads plus
    its partial o-projection; a per-batch ReduceScatter (over tokens) completes
    the sum, leaving core c with tokens [128c, 128c+128) of each batch.
  - Norms + residuals run on the token shard (sequence-parallel).
  - MLP: tensor-parallel over the hidden dim (core c owns HID slice
    [1024c, 1024c+1024)). An AllGather of the (transposed) h2 shard feeds the
    gate/up matmuls; a second per-batch ReduceScatter completes down-proj.
  - Host concatenates the 8 per-core [256, D] output shards.

Matmuls run as float32r (full-rate fp32 variant, ~1.6e-4 component error);
everything else is fp32.
"""
import sys

sys.path.insert(0, "/opt/trn_rl_repo")

import numpy as np

import concourse.bass as bass
import concourse.mybir as mybir
import concourse.tile as tile
from concourse import bacc

F32 = mybir.dt.float32
F32R = mybir.dt.float32r
AF = mybir.ActivationFunctionType
OP = mybir.AluOpType

B, T, D = 2, 1024, 2048
NQ, KV, H, HID = 16, 8, 128, 8192
WINDOW, CAP = 512, 50.0
KMASK = -2.3819763e38
EPS = 1e-6
ROPE_BASE = 10000.0
NCORES = 8
DT = D // 128          # 16 contraction tiles over D
TB = T // 128          # 8 token blocks per batch
RG = [list(range(NCORES))]


def _midx(qb, kb):
    """Canonical additive-mask tile index for (query block, key block)."""
    if kb > qb:
        return 0       # future block: fully masked
    if kb == qb:
        return 1       # causal lower-tri (incl diag)
    if kb == qb - 4:
        return 3       # window tail: strict upper-tri allowed
    if kb < qb - 4:
        return 0       # fully outside window
    return 2           # fully inside window: no mask


def _rms(nc, pools, in_ap, out_ap, mul_bc, width, sq_tile):
    """out = in * rsqrt(mean(in^2)+EPS) * mul   (mul broadcast tile)."""
    ss = pools["small"].tile([128, 1], F32)
    nc.scalar.activation(sq_tile, in_ap, AF.Square, accum_out=ss[:])
    rs = pools["small"].tile([128, 1], F32)
    nc.scalar.activation(rs[:], ss[:], AF.Sqrt, scale=1.0 / width,
                         bias=pools["eps"][:])
    nc.vector.reciprocal(rs[:], rs[:])
    nc.vector.scalar_tensor_tensor(out=out_ap, in0=in_ap, scalar=rs[:],
                                   in1=mul_bc, op0=OP.mult, op1=OP.mult)


def _bcast_row(nc, dst, src_ap):
    """DMA a [W] dram vector broadcast to a [P, W] sbuf tile."""
    nc.sync.dma_start(dst, bass.AP(
        tensor=src_ap.tensor, offset=src_ap.offset,
        ap=[[0, dst.shape[0]], *src_ap.ap]))


def _build_program(reps=1):
    nc = bacc.Bacc("TRN2", target_bir_lowering=False, debug=False,
                   enable_asserts=True, num_devices=NCORES)

    def din(name, shape, dt=F32):
        return nc.dram_tensor(name, shape, dt, kind="ExternalInput").ap()

    x = din("x", [B * T, D])
    xsh = din("xsh", [2 * 128, D])
    wqkv = din("wqkv", [D, 512], F32R)          # [D, 2H q | H k | H v]
    ow = din("ow", [256, D], F32R)              # [2*H rows, D]
    gw = din("gw", [D, 1024], F32R)
    uw = din("uw", [D, 1024], F32R)
    dw = din("dw", [1024, D], F32R)
    cosb = din("cosb", [B * T, 64])
    sinb = din("sinb", [B * T, 64])
    maskb = din("maskb", [4, 128, 128])
    premul = din("premul", [D])
    postattnmul = din("postattnmul", [D])
    preffwmul = din("preffwmul", [D])
    postffwmul = din("postffwmul", [D])
    qmul = din("qmul", [H])
    kmul = din("kmul", [H])
    iden = din("iden", [128, 128], F32R)

    out = nc.dram_tensor("out", [2 * 128, D], F32, kind="ExternalOutput").ap()

    with tile.TileContext(nc) as tc:
        for _ in range(reps):
            _body(nc, tc, x=x, xsh=xsh, wqkv=wqkv, ow=ow, gw=gw, uw=uw,
                  dw=dw, cosb=cosb, sinb=sinb, maskb=maskb, premul=premul,
                  postattnmul=postattnmul, preffwmul=preffwmul,
                  postffwmul=postffwmul, qmul=qmul, kmul=kmul, iden=iden,
                  out=out)
    nc.compile()
    return nc


def _body(nc, tc, *, x, xsh, wqkv, ow, gw, uw, dw, cosb, sinb, maskb, premul,
          postattnmul, preffwmul, postffwmul, qmul, kmul, iden, out):
    from contextlib import ExitStack

    est = ExitStack()
    with est:
        # ----- long-lived pools -----
        consts = est.enter_context(tc.tile_pool(name="consts", bufs=1))
        bcast = est.enter_context(tc.tile_pool(name="bcast", bufs=2))
        small = est.enter_context(tc.tile_pool(name="small", bufs=8))
        obp = est.enter_context(tc.tile_pool(name="obp", bufs=2))
        bwork = est.enter_context(tc.tile_pool(name="bwork", bufs=3))
        dram = est.enter_context(tc.tile_pool(name="dram", bufs=1,
                                              space="DRAM"))
        psA = est.enter_context(tc.tile_pool(name="psA", bufs=3, space="PSUM"))
        psT = est.enter_context(tc.tile_pool(name="psT", bufs=2, space="PSUM"))

        iden_sb = consts.tile([128, 128], F32R)
        nc.sync.dma_start(iden_sb[:], iden[:])
        qmul_bc = consts.tile([128, H], F32)
        _bcast_row(nc, qmul_bc[:], qmul)
        kmul_bc = consts.tile([128, H], F32)
        _bcast_row(nc, kmul_bc[:], kmul)
        eps_t = consts.tile([128, 1], F32)
        nc.vector.memset(eps_t[:], EPS)
        pools = {"small": small, "eps": eps_t}

        premul_bc = bcast.tile([128, D], F32, tag="bc")
        _bcast_row(nc, premul_bc[:], premul)

        # DRAM intermediates
        attn_out_d = dram.tile([B * 128, D], F32)
        o_part = dram.tile([B * T, D], F32)
        rs1 = dram.tile([B * 128, D], F32)
        ag_in = dram.tile([B * D, 128], F32R)
        ag_out = dram.tile([B * NCORES * D, 128], F32R)
        mlp_part = dram.tile([B * T, D], F32)
        rs2 = dram.tile([B * 128, D], F32)

        # =================== ATTENTION (TP over heads) ===================
        with ExitStack() as attn_scope:
            ap_ = {}
            for nm, args in [
                ("csin", dict(bufs=2)), ("maskp", dict(bufs=1)),
                ("wqkvp", dict(bufs=1)), ("owp", dict(bufs=1)),
                ("hT", dict(bufs=1)), ("hp", dict(bufs=1)),
                ("sqs", dict(bufs=1)), ("nrm", dict(bufs=2)),
                ("ro", dict(bufs=2)), ("t64", dict(bufs=2)),
                ("qT", dict(bufs=1)), ("kT", dict(bufs=1)),
                ("vp", dict(bufs=1)), ("t1", dict(bufs=1)),
                ("exp", dict(bufs=1)), ("prob", dict(bufs=1)),
                ("pT", dict(bufs=1)), ("encp", dict(bufs=2)),
                ("bh2", dict(bufs=1)),
            ]:
                ap_[nm] = attn_scope.enter_context(
                    tc.tile_pool(name=nm, **args))
            psL = attn_scope.enter_context(
                tc.tile_pool(name="psL", bufs=1, space="PSUM"))

            maskb_sb = ap_["maskp"].tile([128, 4, 128], F32)
            nc.sync.dma_start(maskb_sb[:], maskb.rearrange("m p k -> p m k"))
            wqkv_sb = ap_["wqkvp"].tile([128, DT, 512], F32R)
            nc.sync.dma_start(wqkv_sb[:],
                              wqkv.rearrange("(dt p) c -> p dt c", p=128))
            ow_sb = ap_["owp"].tile([128, 2, 4, 512], F32R)
            for hh in range(2):
                for ch in range(4):
                    nc.sync.dma_start(
                        ow_sb[:, hh, ch, :],
                        ow[hh * 128:(hh + 1) * 128, ch * 512:(ch + 1) * 512])

            for b in range(B):
                cos_t = ap_["csin"].tile([128, TB, 64], F32, tag="cs")
                nc.sync.dma_start(
                    cos_t[:], cosb[b * T:(b + 1) * T, :].rearrange(
                        "(tb p) h -> p tb h", p=128))
                sin_t = ap_["csin"].tile([128, TB, 64], F32, tag="cs")
                nc.sync.dma_start(
                    sin_t[:], sinb[b * T:(b + 1) * T, :].rearrange(
                        "(tb p) h -> p tb h", p=128))

                qT = ap_["qT"].tile([128, 2, T], F32R)
                kT = ap_["kT"].tile([128, T], F32R)
                v_sb = ap_["vp"].tile([128, TB, 128], F32R)

                for hf in range(4):       # quarter-batch over tokens
                    hT = ap_["hT"].tile([128, DT, 256], F32R)
                    # ---- h = rms(x)*premul, transposed into hT ----
                    for t4 in range(2):
                        tb = hf * 2 + t4
                        xt = bwork.tile([128, D], F32, tag="bw")
                        nc.sync.dma_start(
                            xt[:], x[b * T + tb * 128: b * T + (tb + 1) * 128, :])
                        ht = ap_["hp"].tile([128, D], F32R)
                        sq = bwork.tile([128, D], F32, tag="bw")
                        _rms(nc, pools, xt[:], ht[:], premul_bc[:], D, sq[:])
                        for dt in range(DT):
                            pt = psT.tile([128, 128], F32R)
                            nc.tensor.transpose(
                                pt[:], ht[:, dt * 128:(dt + 1) * 128],
                                iden_sb[:])
                            nc.vector.tensor_copy(
                                hT[:, dt, t4 * 128:(t4 + 1) * 128], pt[:])
                    # ---- qkv projections + qk-norm + rope ----
                    for t4 in range(2):
                        tb = hf * 2 + t4
                        pq = psA.tile([128, 512], F32, tag="mm")
                        for dt in range(DT):
                            nc.tensor.matmul(
                                pq[:], hT[:, dt, t4 * 128:(t4 + 1) * 128],
                                wqkv_sb[:, dt, :],
                                start=(dt == 0), stop=(dt == DT - 1))
                        for hd in range(3):    # q0, q1, k
                            sl = pq[:, hd * 128:(hd + 1) * 128]
                            nrm = ap_["nrm"].tile([128, 128], F32)
                            sqs = ap_["sqs"].tile([128, 128], F32)
                            _rms(nc, pools, sl, nrm[:],
                                 (qmul_bc if hd < 2 else kmul_bc)[:], H,
                                 sqs[:])
                            ro = ap_["ro"].tile([128, 128], F32R)
                            ct, st = cos_t[:, tb, :], sin_t[:, tb, :]
                            t1 = ap_["t64"].tile([128, 64], F32)
                            t2 = ap_["t64"].tile([128, 64], F32)
                            nc.vector.tensor_mul(t1[:], nrm[:, 0:64], ct)
                            nc.vector.tensor_mul(t2[:], nrm[:, 64:128], st)
                            nc.vector.tensor_sub(ro[:, 0:64], t1[:], t2[:])
                            t3 = ap_["t64"].tile([128, 64], F32)
                            t4_ = ap_["t64"].tile([128, 64], F32)
                            nc.vector.tensor_mul(t3[:], nrm[:, 64:128], ct)
                            nc.vector.tensor_mul(t4_[:], nrm[:, 0:64], st)
                            nc.vector.tensor_add(ro[:, 64:128], t3[:], t4_[:])
                            pt = psT.tile([128, 128], F32R)
                            nc.tensor.transpose(pt[:], ro[:], iden_sb[:])
                            dst = (qT[:, hd, tb * 128:(tb + 1) * 128]
                                   if hd < 2 else
                                   kT[:, tb * 128:(tb + 1) * 128])
                            nc.vector.tensor_copy(dst, pt[:])
                        nc.vector.tensor_copy(v_sb[:, tb, :], pq[:, 384:512])

                # ---- banded attention, query-block pairs ----
                for p in range(4):
                    ukb0 = max(0, 2 * p - 4)
                    nu = 2 * p + 2 - ukb0
                    keys = nu * 128
                    enc = ap_["encp"].tile([128, 2, 256], F32R)
                    for h in range(2):
                        probsT = ap_["pT"].tile([128, 6, 256], F32R)
                        for qi in range(2):
                            qb = 2 * p + qi
                            psl = psL.tile([128, 768], F32)
                            c0 = 0
                            while c0 < keys:
                                c1 = min(c0 + 512, keys)
                                nc.tensor.matmul(
                                    psl[:, c0:c1],
                                    qT[:, h, qb * 128:(qb + 1) * 128],
                                    kT[:, ukb0 * 128 + c0: ukb0 * 128 + c1],
                                    start=True, stop=True)
                                c0 = c1
                            t1 = ap_["t1"].tile([128, 768], F32)
                            nc.scalar.activation(t1[:, 0:keys], psl[:, 0:keys],
                                                 AF.Tanh, scale=1.0 / CAP)
                            for j in range(nu):
                                mi = _midx(qb, ukb0 + j)
                                nc.vector.scalar_tensor_tensor(
                                    out=t1[:, j * 128:(j + 1) * 128],
                                    in0=t1[:, j * 128:(j + 1) * 128],
                                    scalar=CAP, in1=maskb_sb[:, mi, :],
                                    op0=OP.mult, op1=OP.add)
                            ex = ap_["exp"].tile([128, 768], F32)
                            den = small.tile([128, 1], F32)
                            nc.scalar.activation(ex[:, 0:keys], t1[:, 0:keys],
                                                 AF.Exp, accum_out=den[:])
                            nc.vector.reciprocal(den[:], den[:])
                            prob = ap_["prob"].tile([128, 768], F32R)
                            nc.vector.tensor_scalar_mul(
                                prob[:, 0:keys], ex[:, 0:keys], den[:])
                            for j in range(nu):
                                pt = psT.tile([128, 128], F32R)
                                nc.tensor.transpose(
                                    pt[:], prob[:, j * 128:(j + 1) * 128],
                                    iden_sb[:])
                                nc.vector.tensor_copy(
                                    probsT[:, j, qi * 128:(qi + 1) * 128],
                                    pt[:])
                        pav = psA.tile([128, 256], F32, tag="mm")
                        for j in range(nu):
                            nc.tensor.matmul(
                                pav[:], v_sb[:, ukb0 + j, :], probsT[:, j, :],
                                start=(j == 0), stop=(j == nu - 1))
                        nc.vector.tensor_copy(enc[:, h, :], pav[:])
                    # o-projection (partial over this core's 2 heads)
                    for qi in range(2):
                        qb = 2 * p + qi
                        for ch in range(4):
                            po = psA.tile([128, 512], F32, tag="mm")
                            for h in range(2):
                                nc.tensor.matmul(
                                    po[:], enc[:, h, qi * 128:(qi + 1) * 128],
                                    ow_sb[:, h, ch, :],
                                    start=(h == 0), stop=(h == 1))
                            ob = obp.tile([128, 512], F32)
                            nc.vector.tensor_copy(ob[:], po[:])
                            nc.sync.dma_start(
                                o_part[b * T + qb * 128: b * T + (qb + 1) * 128,
                                       ch * 512:(ch + 1) * 512], ob[:])

                nc.gpsimd.collective_compute(
                    "ReduceScatter", OP.add, replica_groups=RG,
                    ins=[o_part[b * T:(b + 1) * T, :].opt()],
                    outs=[rs1[b * 128:(b + 1) * 128, :].opt()])

            # ---------- sequence-parallel norm chain + AllGather ----------
            postattn_bc = bcast.tile([128, D], F32, tag="bc")
            _bcast_row(nc, postattn_bc[:], postattnmul)
            preffw_bc = bcast.tile([128, D], F32, tag="bc")
            _bcast_row(nc, preffw_bc[:], preffwmul)
            for b in range(B):
                rst = bwork.tile([128, D], F32, tag="bw")
                nc.sync.dma_start(rst[:], rs1[b * 128:(b + 1) * 128, :])
                ao = bwork.tile([128, D], F32, tag="bw")
                _rms(nc, pools, rst[:], ao[:], postattn_bc[:], D, ao[:])
                xst = bwork.tile([128, D], F32, tag="bw")
                nc.sync.dma_start(xst[:], xsh[b * 128:(b + 1) * 128, :])
                nc.vector.tensor_add(ao[:], ao[:], xst[:])
                nc.sync.dma_start(attn_out_d[b * 128:(b + 1) * 128, :], ao[:])
                h2 = ap_["bh2"].tile([128, D], F32R)
                sq2 = bwork.tile([128, D], F32, tag="bw")
                _rms(nc, pools, ao[:], h2[:], preffw_bc[:], D, sq2[:])
                h2T = ap_["bh2"].tile([128, DT, 128], F32R, tag="h2T")
                for dt in range(DT):
                    pt = psT.tile([128, 128], F32R)
                    nc.tensor.transpose(pt[:], h2[:, dt * 128:(dt + 1) * 128],
                                        iden_sb[:])
                    nc.vector.tensor_copy(h2T[:, dt, :], pt[:])
                nc.sync.dma_start(
                    ag_in[b * D:(b + 1) * D, :].rearrange(
                        "(dt p) c -> p dt c", p=128), h2T[:])
                nc.gpsimd.collective_compute(
                    "AllGather", OP.bypass, replica_groups=RG,
                    ins=[ag_in[b * D:(b + 1) * D, :].opt()],
                    outs=[ag_out[b * NCORES * D:(b + 1) * NCORES * D, :].opt()])

        # =================== MLP (TP over hidden dim) ===================
        with ExitStack() as mlp_scope:
            mp_ = {}
            for nm, args in [
                ("h2Tp", dict(bufs=1)), ("wst", dict(bufs=2)),
                ("actp", dict(bufs=1)), ("gel", dict(bufs=2)),
                ("dwp", dict(bufs=2)),
            ]:
                mp_[nm] = mlp_scope.enter_context(tc.tile_pool(name=nm, **args))

            for b in range(B):
                h2Tf = mp_["h2Tp"].tile([128, DT, 1024], F32R)
                base = b * NCORES * D
                for r in range(NCORES):
                    nc.sync.dma_start(
                        h2Tf[:, :, r * 128:(r + 1) * 128],
                        ag_out[base + r * D: base + (r + 1) * D, :].rearrange(
                            "(dt p) c -> p dt c", p=128))
                actT = mp_["actp"].tile([128, 8, 1024], F32R)
                for hc in range(8):
                    gw_t = mp_["wst"].tile([128, DT, 128], F32R, tag="w")
                    nc.sync.dma_start(
                        gw_t[:], gw[:, hc * 128:(hc + 1) * 128].rearrange(
                            "(dt p) h -> p dt h", p=128))
                    uw_t = mp_["wst"].tile([128, DT, 128], F32R, tag="w")
                    nc.sync.dma_start(
                        uw_t[:], uw[:, hc * 128:(hc + 1) * 128].rearrange(
                            "(dt p) h -> p dt h", p=128))
                    for tch in range(2):
                        psg = psA.tile([128, 512], F32, tag="mm")
                        psu = psA.tile([128, 512], F32, tag="mm")
                        for dt in range(DT):
                            nc.tensor.matmul(
                                psg[:], gw_t[:, dt, :],
                                h2Tf[:, dt, tch * 512:(tch + 1) * 512],
                                start=(dt == 0), stop=(dt == DT - 1))
                        for dt in range(DT):
                            nc.tensor.matmul(
                                psu[:], uw_t[:, dt, :],
                                h2Tf[:, dt, tch * 512:(tch + 1) * 512],
                                start=(dt == 0), stop=(dt == DT - 1))
                        gel = mp_["gel"].tile([128, 512], F32)
                        nc.scalar.activation(gel[:], psg[:],
                                             AF.Gelu_apprx_tanh)
                        nc.vector.tensor_mul(
                            actT[:, hc, tch * 512:(tch + 1) * 512],
                            gel[:], psu[:])
                # down-projection, streamed per 512-wide D chunk
                for ch in range(4):
                    dw_t = mp_["dwp"].tile([128, 8, 512], F32R)
                    for hc in range(8):
                        nc.sync.dma_start(
                            dw_t[:, hc, :],
                            dw[hc * 128:(hc + 1) * 128,
                               ch * 512:(ch + 1) * 512])
                    for tb in range(TB):
                        psd = psA.tile([128, 512], F32, tag="mm")
                        for hc in range(8):
                            nc.tensor.matmul(
                                psd[:], actT[:, hc, tb * 128:(tb + 1) * 128],
                                dw_t[:, hc, :],
                                start=(hc == 0), stop=(hc == 7))
                        mb = obp.tile([128, 512], F32)
                        nc.vector.tensor_copy(mb[:], psd[:])
                        nc.sync.dma_start(
                            mlp_part[b * T + tb * 128: b * T + (tb + 1) * 128,
                                     ch * 512:(ch + 1) * 512], mb[:])
                nc.gpsimd.collective_compute(
                    "ReduceScatter", OP.add, replica_groups=RG,
                    ins=[mlp_part[b * T:(b + 1) * T, :].opt()],
                    outs=[rs2[b * 128:(b + 1) * 128, :].opt()])

        # =================== final norm + residual ===================
        postffw_bc = bcast.tile([128, D], F32, tag="bc")
        _bcast_row(nc, postffw_bc[:], postffwmul)
        for b in range(B):
            rst = bwork.tile([128, D], F32, tag="bw")
            nc.sync.dma_start(rst[:], rs2[b * 128:(b + 1) * 128, :])
            tmp = bwork.tile([128, D], F32, tag="bw")
            _rms(nc, pools, rst[:], tmp[:], postffw_bc[:], D, tmp[:])
            aol = bwork.tile([128, D], F32, tag="bw")
            nc.sync.dma_start(aol[:], attn_out_d[b * 128:(b + 1) * 128, :])
            nc.vector.tensor_add(tmp[:], tmp[:], aol[:])
            nc.sync.dma_start(out[b * 128:(b + 1) * 128, :], tmp[:])


# ---------------------------------------------------------------------------
# host side
# ---------------------------------------------------------------------------

_NC = None


def _get_nc():
    global _NC
    if _NC is None:
        _NC = _build_program()
    return _NC


def _host_prep(inputs):
    """Build the 8 per-core input maps from the full problem inputs."""
    x = np.ascontiguousarray(np.asarray(inputs["x"], dtype=np.float32))
    seg = np.asarray(inputs["segment_pos"], dtype=np.int32)
    am = np.asarray(inputs["attn_mask"])
    q_k = np.asarray(inputs["q_kernel"], dtype=np.float32)
    kv_k = np.asarray(inputs["kv_kernel"], dtype=np.float32)
    o_k = np.asarray(inputs["o_kernel"], dtype=np.float32)
    gate_w = np.asarray(inputs["gate_w"], dtype=np.float32)
    up_w = np.asarray(inputs["up_w"], dtype=np.float32)
    down_w = np.asarray(inputs["down_w"], dtype=np.float32)

    xf = x.reshape(B * T, D)
    premul = (1.0 + np.asarray(inputs["pre_attn_scale"], np.float32))
    postattn = (1.0 + np.asarray(inputs["post_attn_scale"], np.float32))
    preffw = (1.0 + np.asarray(inputs["pre_ffw_scale"], np.float32))
    postffw = (1.0 + np.asarray(inputs["post_ffw_scale"], np.float32))
    qmul = ((1.0 + np.asarray(inputs["q_norm_scale"], np.float32))
            * np.float32(H ** -0.5)).astype(np.float32)
    kmul = (1.0 + np.asarray(inputs["k_norm_scale"], np.float32))

    frac = (2.0 * np.arange(H // 2, dtype=np.float32) / H).astype(np.float32)
    ts = (ROPE_BASE ** frac).astype(np.float32)
    sinu = (seg[..., None].astype(np.float32) / ts).astype(np.float32)
    cosb = np.cos(sinu).reshape(B * T, 64).astype(np.float32)
    sinb = np.sin(sinu).reshape(B * T, 64).astype(np.float32)

    ti = np.arange(128)[:, None]
    si = np.arange(128)[None, :]
    maskb = np.stack([
        np.full((128, 128), KMASK, np.float32),
        np.where(ti >= si, 0.0, KMASK).astype(np.float32),
        np.zeros((128, 128), np.float32),
        np.where(ti < si, 0.0, KMASK).astype(np.float32),
    ]).astype(np.float32)

    # soft structural check of the actual mask against the canonical tiles
    tt = np.arange(T)
    sliding = (np.abs(tt[:, None] - tt[None, :]) <= WINDOW - 1)
    expected = am & sliding[None]
    ok = True
    for qb in range(min(2, TB)):
        for kb in range(qb + 1):
            blk = np.where(expected[0, qb * 128:(qb + 1) * 128,
                                    kb * 128:(kb + 1) * 128], 0.0, KMASK)
            if not np.array_equal(blk.astype(np.float32),
                                  maskb[_midx(qb, kb)]):
                ok = False
    if not ok:
        print("kernel.py WARNING: attn_mask does not match canonical "
              "causal+sliding structure; results may be wrong")

    iden = np.eye(128, dtype=np.float32)

    in_maps = []
    for c in range(NCORES):
        qw_c = q_k[2 * c:2 * c + 2].transpose(1, 0, 2).reshape(D, 256)
        kw_c = kv_k[0, c]
        vw_c = kv_k[1, c]
        wqkv_c = np.ascontiguousarray(
            np.concatenate([qw_c, kw_c, vw_c], axis=1), dtype=np.float32)
        ow_c = np.ascontiguousarray(
            o_k[2 * c:2 * c + 2].reshape(256, D), dtype=np.float32)
        gw_c = np.ascontiguousarray(gate_w[:, 1024 * c:1024 * (c + 1)])
        uw_c = np.ascontiguousarray(up_w[:, 1024 * c:1024 * (c + 1)])
        dw_c = np.ascontiguousarray(down_w[1024 * c:1024 * (c + 1), :])
        xsh_c = np.ascontiguousarray(np.concatenate(
            [xf[128 * c:128 * (c + 1)],
             xf[T + 128 * c: T + 128 * (c + 1)]], axis=0))
        in_maps.append({
            "x": xf, "xsh": xsh_c, "wqkv": wqkv_c, "ow": ow_c,
            "gw": gw_c, "uw": uw_c, "dw": dw_c,
            "cosb": cosb, "sinb": sinb, "maskb": maskb,
            "premul": premul, "postattnmul": postattn,
            "preffwmul": preffw, "postffwmul": postffw,
            "qmul": qmul, "kmul": kmul, "iden": iden,
        })
    return in_maps


def _assemble(results):
    out = np.empty((B, T, D), dtype=np.float32)
    for c in range(NCORES):
        r = results[c]["out"]
        out[0, 128 * c:128 * (c + 1)] = r[0:128]
        out[1, 128 * c:128 * (c + 1)] = r[128:256]
    return out


def kernel(**inputs) -> np.ndarray:
    from concourse import bass_utils
    nc = _get_nc()
    in_maps = _host_prep(inputs)
    r = bass_utils.run_bass_kernel_spmd(nc, in_maps,
                                        core_ids=list(range(NCORES)))
    return _assemble(r.results)


# revision 60
# speedup vs baseline: 1.3139x; 1.0672x over previous
"""Gemma-style transformer block (GQA + sliding-window attention + gated-GELU
MLP) on 8 Trainium2 NeuronCores.

Sharding (Megatron + sequence-parallel):
  - Attention: tensor-parallel over heads. Core c owns q heads {2c, 2c+1} and
    kv head c. Each core computes full-sequence attention for its heads plus
    its partial o-projection; a per-batch ReduceScatter (over tokens) completes
    the sum, leaving core c with tokens [128c, 128c+128) of each batch.
  - Norms + residuals run on the token shard (sequence-parallel).
  - MLP: tensor-parallel over the hidden dim (core c owns HID slice
    [1024c, 1024c+1024)). An AllGather of the (transposed) h2 shard feeds the
    gate/up matmuls; a second per-batch ReduceScatter completes down-proj.
  - Host concatenates the 8 per-core [256, D] output shards.

Matmuls run as float32r (full-rate fp32 variant, ~1.6e-4 component error);
everything else is fp32.
"""
import sys

sys.path.insert(0, "/opt/trn_rl_repo")

import numpy as np

import concourse.bass as bass
import concourse.mybir as mybir
import concourse.tile as tile
from concourse import bacc

F32 = mybir.dt.float32
F32R = mybir.dt.float32r
AF = mybir.ActivationFunctionType
OP = mybir.AluOpType

B, T, D = 2, 1024, 2048
NQ, KV, H, HID = 16, 8, 128, 8192
WINDOW, CAP = 512, 50.0
KMASK = -2.3819763e38
EPS = 1e-6
ROPE_BASE = 10000.0
NCORES = 8
DT = D // 128          # 16 contraction tiles over D
TB = T // 128          # 8 token blocks per batch
RG = [list(range(NCORES))]

TUNE = {"psA": 2, "psL": 2, "psT": 2, "pT": 2, "hT": 3, "prob": 1,
        "psM": 3, "encp": 2}


def _midx(qb, kb):
    """Canonical additive-mask tile index for (query block, key block)."""
    if kb > qb:
        return 0       # future block: fully masked
    if kb == qb:
        return 1       # causal lower-tri (incl diag)
    if kb == qb - 4:
        return 3       # window tail: strict upper-tri allowed
    if kb < qb - 4:
        return 0       # fully outside window
    return 2           # fully inside window: no mask


def _rms(nc, pools, in_ap, out_ap, mul_bc, width, sq_tile):
    """out = in * rsqrt(mean(in^2)+EPS) * mul   (mul broadcast tile)."""
    ss = pools["small"].tile([128, 1], F32)
    nc.scalar.activation(sq_tile, in_ap, AF.Square, accum_out=ss[:])
    rs = pools["small"].tile([128, 1], F32)
    nc.scalar.activation(rs[:], ss[:], AF.Sqrt, scale=1.0 / width,
                         bias=pools["eps"][:])
    nc.vector.reciprocal(rs[:], rs[:])
    nc.vector.scalar_tensor_tensor(out=out_ap, in0=in_ap, scalar=rs[:],
                                   in1=mul_bc, op0=OP.mult, op1=OP.mult)


def _dma4(nc, dst, src, n=4):
    """Split a [128, W] (or matching) DMA into n column chunks across queues."""
    W = dst.shape[-1]
    step = W // n
    for i in range(n):
        sl = (slice(None),) * (len(dst.shape) - 1)
        nc.sync.dma_start(dst[(*sl, slice(i * step, (i + 1) * step))],
                          src[(*(slice(None),) * (len(src.shape) - 1),
                               slice(i * step, (i + 1) * step))])


def _bcast_row(nc, dst, src_ap):
    """DMA a [W] dram vector broadcast to a [P, W] sbuf tile."""
    nc.sync.dma_start(dst, bass.AP(
        tensor=src_ap.tensor, offset=src_ap.offset,
        ap=[[0, dst.shape[0]], *src_ap.ap]))


def _build_program(reps=1, single=False, do_attn=True, do_mlp=True,
                   fake_coll=False, stop_after=None, fake_rs1=False,
                   fake_ag=False, fake_rs2=False):
    nc = bacc.Bacc("TRN2", target_bir_lowering=False, debug=False,
                   enable_asserts=True,
                   num_devices=(1 if single else NCORES))

    def din(name, shape, dt=F32):
        return nc.dram_tensor(name, shape, dt, kind="ExternalInput").ap()

    x = din("x", [B * T, D])
    xsh = din("xsh", [2 * 128, D])
    wqkv = din("wqkv", [D, 512], F32R)          # [D, 2H q | H k | H v]
    ow = din("ow", [256, D], F32R)              # [2*H rows, D]
    gw = din("gw", [D, 1024], F32R)
    uw = din("uw", [D, 1024], F32R)
    dw = din("dw", [1024, D], F32R)
    cosb = din("cosb", [B * T, 64])
    sinb = din("sinb", [B * T, 64])
    maskb = din("maskb", [4, 128, 128])
    premul = din("premul", [D])
    postattnmul = din("postattnmul", [D])
    preffwmul = din("preffwmul", [D])
    postffwmul = din("postffwmul", [D])
    qmul = din("qmul", [H])
    kmul = din("kmul", [H])
    iden = din("iden", [128, 128], F32R)

    out = nc.dram_tensor("out", [2 * 128, D], F32, kind="ExternalOutput").ap()

    with tile.TileContext(nc) as tc:
        for _ in range(reps):
            _body(nc, tc, x=x, xsh=xsh, wqkv=wqkv, ow=ow, gw=gw, uw=uw,
                  dw=dw, cosb=cosb, sinb=sinb, maskb=maskb, premul=premul,
                  postattnmul=postattnmul, preffwmul=preffwmul,
                  postffwmul=postffwmul, qmul=qmul, kmul=kmul, iden=iden,
                  out=out, single=single, do_attn=do_attn, do_mlp=do_mlp,
                  fake_coll=fake_coll, stop_after=stop_after,
                  fake_rs1=fake_rs1, fake_ag=fake_ag, fake_rs2=fake_rs2)
    nc.compile()
    return nc


def _body(nc, tc, *, x, xsh, wqkv, ow, gw, uw, dw, cosb, sinb, maskb, premul,
          postattnmul, preffwmul, postffwmul, qmul, kmul, iden, out,
          single=False, do_attn=True, do_mlp=True, fake_coll=False,
          stop_after=None, fake_rs1=False, fake_ag=False, fake_rs2=False):
    from contextlib import ExitStack

    if single:
        fake_coll = True
    if fake_coll:
        fake_rs1 = fake_ag = fake_rs2 = True

    class _Stop(Exception):
        pass

    def _ckpt(name):
        if stop_after == name:
            raise _Stop()

    def _rscatter(in_ap, out_ap, fake):
        if fake:
            nrows = out_ap.shape[0]
            nc.sync.dma_start(out_ap, in_ap[0:nrows, :])
        else:
            nc.gpsimd.collective_compute(
                "ReduceScatter", OP.add, replica_groups=RG,
                ins=[in_ap.opt()], outs=[out_ap.opt()])

    def _agather(in_ap, out_ap):
        if fake_ag:
            nrows = in_ap.shape[0]
            for r in range(NCORES):
                nc.sync.dma_start(out_ap[r * nrows:(r + 1) * nrows, :], in_ap)
        else:
            nc.gpsimd.collective_compute(
                "AllGather", OP.bypass, replica_groups=RG,
                ins=[in_ap.opt()], outs=[out_ap.opt()])

    try:
        _body_inner(nc, tc, _Stop=_Stop, _ckpt=_ckpt, _rscatter=_rscatter,
                    _agather=_agather, x=x, xsh=xsh, wqkv=wqkv, ow=ow, gw=gw,
                    uw=uw, dw=dw, cosb=cosb, sinb=sinb, maskb=maskb,
                    premul=premul, postattnmul=postattnmul,
                    preffwmul=preffwmul, postffwmul=postffwmul, qmul=qmul,
                    kmul=kmul, iden=iden, out=out, single=single,
                    do_attn=do_attn, do_mlp=do_mlp, fake_coll=fake_coll,
                    fake_rs1=fake_rs1, fake_ag=fake_ag, fake_rs2=fake_rs2)
    except _Stop:
        pass


def _body_inner(nc, tc, *, _Stop, _ckpt, _rscatter, _agather, x, xsh, wqkv,
                ow, gw, uw, dw, cosb, sinb, maskb, premul, postattnmul,
                preffwmul, postffwmul, qmul, kmul, iden, out, single, do_attn,
                do_mlp, fake_coll, fake_rs1=False, fake_ag=False,
                fake_rs2=False):
    from contextlib import ExitStack

    est = ExitStack()
    with est:
        # ----- long-lived pools -----
        consts = est.enter_context(tc.tile_pool(name="consts", bufs=1))
        bcast = est.enter_context(tc.tile_pool(name="bcast", bufs=3))
        small = est.enter_context(tc.tile_pool(name="small", bufs=8))
        obp = est.enter_context(tc.tile_pool(name="obp", bufs=3))
        bwork = est.enter_context(tc.tile_pool(name="bwork", bufs=5))
        dram = est.enter_context(tc.tile_pool(name="dram", bufs=1,
                                              space="DRAM"))
        psA = est.enter_context(tc.tile_pool(name="psA", bufs=TUNE["psA"], space="PSUM"))
        psT = est.enter_context(tc.tile_pool(name="psT", bufs=TUNE["psT"], space="PSUM"))

        iden_sb = consts.tile([128, 128], F32R)
        nc.sync.dma_start(iden_sb[:], iden[:])
        qmul_bc = consts.tile([128, H], F32)
        _bcast_row(nc, qmul_bc[:], qmul)
        kmul_bc = consts.tile([128, H], F32)
        _bcast_row(nc, kmul_bc[:], kmul)
        eps_t = consts.tile([128, 1], F32)
        nc.vector.memset(eps_t[:], EPS)
        pools = {"small": small, "eps": eps_t}

        premul_bc = bcast.tile([128, D], F32, tag="bc")
        _bcast_row(nc, premul_bc[:], premul)
        postattn_bc = bcast.tile([128, D], F32, tag="bc")
        _bcast_row(nc, postattn_bc[:], postattnmul)
        preffw_bc = bcast.tile([128, D], F32, tag="bc")
        _bcast_row(nc, preffw_bc[:], preffwmul)

        # DRAM intermediates
        attn_out_d = dram.tile([B * 128, D], F32)
        o_part = dram.tile([B * T, D], F32)
        rs1 = dram.tile([B * 128, D], F32)
        ag_in = dram.tile([B * D, 128], F32R)
        ag_sp = "Local" if fake_ag else "Shared"
        ag_outs = [
            dram.tile([NCORES * D, 128], F32R, addr_space=ag_sp,
                      tag=f"ag_out{b}", name=f"ag_out{b}")
            for b in range(B)
        ]
        mlp_part = dram.tile([B * T, D], F32)
        rs2 = dram.tile([B * 128, D], F32)

        def _bchain(b):
            """post-attn norm + residual + pre-ffw norm + h2T + AllGather,
            on this core's token shard of batch b (sequence-parallel)."""
            rst = bwork.tile([128, D], F32, tag="bw", name="rst")
            _dma4(nc, rst[:], rs1[b * 128:(b + 1) * 128, :])
            ao = bwork.tile([128, D], F32, tag="bw", name="ao")
            _rms(nc, pools, rst[:], ao[:], postattn_bc[:], D, ao[:])
            xst = bwork.tile([128, D], F32, tag="bw", name="xst")
            _dma4(nc, xst[:], xsh[b * 128:(b + 1) * 128, :])
            nc.vector.tensor_add(ao[:], ao[:], xst[:])
            _dma4(nc, attn_out_d[b * 128:(b + 1) * 128, :], ao[:])
            h2 = bwork.tile([128, D], F32R, tag="bw", name="h2")
            sq2 = bwork.tile([128, D], F32, tag="bw", name="sq2")
            _rms(nc, pools, ao[:], h2[:], preffw_bc[:], D, sq2[:])
            h2T = bwork.tile([128, DT, 128], F32R, tag="bw", name="h2T")
            for dt in range(DT):
                pt = psT.tile([128, 128], F32R)
                nc.tensor.transpose(pt[:], h2[:, dt * 128:(dt + 1) * 128],
                                    iden_sb[:])
                nc.vector.tensor_copy(h2T[:, dt, :], pt[:])
            agv = ag_in[b * D:(b + 1) * D, :].rearrange(
                "(dt p) c -> p dt c", p=128)
            for g in range(4):
                nc.sync.dma_start(agv[:, g * 4:(g + 1) * 4, :],
                                  h2T[:, g * 4:(g + 1) * 4, :])
            _agather(ag_in[b * D:(b + 1) * D, :], ag_outs[b][:, :])

        # =================== ATTENTION (TP over heads) ===================
        with ExitStack() as attn_scope:
          if do_attn:
            ap_ = {}
            for nm, args in [
                ("csin", dict(bufs=2)), ("maskp", dict(bufs=1)),
                ("wqkvp", dict(bufs=1)), ("owp", dict(bufs=1)),
                ("hT", dict(bufs=TUNE["hT"])), ("hp", dict(bufs=1)),
                ("sqs", dict(bufs=1)), ("nrm", dict(bufs=2)),
                ("ro", dict(bufs=2)), ("t64", dict(bufs=2)),
                ("qT", dict(bufs=1)), ("kT", dict(bufs=1)),
                ("vp", dict(bufs=1)), ("t1", dict(bufs=1)),
                ("exp", dict(bufs=1)), ("prob", dict(bufs=TUNE["prob"])),
                ("pT", dict(bufs=TUNE["pT"])), ("encp", dict(bufs=TUNE["encp"])),
                ("bh2", dict(bufs=1)),
            ]:
                ap_[nm] = attn_scope.enter_context(
                    tc.tile_pool(name=nm, **args))
            psL = attn_scope.enter_context(
                tc.tile_pool(name="psL", bufs=TUNE["psL"], space="PSUM"))

            maskb_sb = ap_["maskp"].tile([128, 4, 128], F32)
            nc.sync.dma_start(maskb_sb[:], maskb.rearrange("m p k -> p m k"))
            wqkv_sb = ap_["wqkvp"].tile([128, DT, 512], F32R)
            for dt in range(DT):
                nc.sync.dma_start(wqkv_sb[:, dt, :],
                                  wqkv[dt * 128:(dt + 1) * 128, :])
            ow_sb = ap_["owp"].tile([128, 2, 4, 512], F32R)
            for hh in range(2):
                for ch in range(4):
                    nc.sync.dma_start(
                        ow_sb[:, hh, ch, :],
                        ow[hh * 128:(hh + 1) * 128, ch * 512:(ch + 1) * 512])

            for b in range(B):
                cos_t = ap_["csin"].tile([128, TB, 64], F32, tag="cs")
                nc.sync.dma_start(
                    cos_t[:], cosb[b * T:(b + 1) * T, :].rearrange(
                        "(tb p) h -> p tb h", p=128))
                sin_t = ap_["csin"].tile([128, TB, 64], F32, tag="cs")
                nc.sync.dma_start(
                    sin_t[:], sinb[b * T:(b + 1) * T, :].rearrange(
                        "(tb p) h -> p tb h", p=128))

                qT = ap_["qT"].tile([128, 2, T], F32R)
                kT = ap_["kT"].tile([128, T], F32R)
                v_sb = ap_["vp"].tile([128, TB, 128], F32R)

                for tb in range(TB):
                    hT = ap_["hT"].tile([128, DT, 128], F32R)
                    # ---- h = rms(x)*premul, transposed into hT ----
                    if True:
                        xt = bwork.tile([128, D], F32, tag="bw")
                        _dma4(nc, xt[:],
                              x[b * T + tb * 128: b * T + (tb + 1) * 128, :])
                        ht = ap_["hp"].tile([128, D], F32R)
                        sq = bwork.tile([128, D], F32, tag="bw")
                        _rms(nc, pools, xt[:], ht[:], premul_bc[:], D, sq[:])
                        for dt in range(DT):
                            pt = psT.tile([128, 128], F32R)
                            nc.tensor.transpose(
                                pt[:], ht[:, dt * 128:(dt + 1) * 128],
                                iden_sb[:])
                            nc.vector.tensor_copy(hT[:, dt, :], pt[:])
                    # ---- qkv projections + qk-norm + rope ----
                    if True:
                        pq = psA.tile([128, 512], F32, tag="mm")
                        for dt in range(DT):
                            nc.tensor.matmul(
                                pq[:], hT[:, dt, :], wqkv_sb[:, dt, :],
                                start=(dt == 0), stop=(dt == DT - 1))
                        for hd in range(3):    # q0, q1, k
                            sl = pq[:, hd * 128:(hd + 1) * 128]
                            nrm = ap_["nrm"].tile([128, 128], F32)
                            sqs = ap_["sqs"].tile([128, 128], F32)
                            _rms(nc, pools, sl, nrm[:],
                                 (qmul_bc if hd < 2 else kmul_bc)[:], H,
                                 sqs[:])
                            ro = ap_["ro"].tile([128, 128], F32R)
                            ct, st = cos_t[:, tb, :], sin_t[:, tb, :]
                            t1 = ap_["t64"].tile([128, 64], F32)
                            t2 = ap_["t64"].tile([128, 64], F32)
                            nc.vector.tensor_mul(t1[:], nrm[:, 0:64], ct)
                            nc.vector.tensor_mul(t2[:], nrm[:, 64:128], st)
                            nc.vector.tensor_sub(ro[:, 0:64], t1[:], t2[:])
                            t3 = ap_["t64"].tile([128, 64], F32)
                            t4_ = ap_["t64"].tile([128, 64], F32)
                            nc.vector.tensor_mul(t3[:], nrm[:, 64:128], ct)
                            nc.vector.tensor_mul(t4_[:], nrm[:, 0:64], st)
                            nc.vector.tensor_add(ro[:, 64:128], t3[:], t4_[:])
                            pt = psT.tile([128, 128], F32R)
                            nc.tensor.transpose(pt[:], ro[:], iden_sb[:])
                            dst = (qT[:, hd, tb * 128:(tb + 1) * 128]
                                   if hd < 2 else
                                   kT[:, tb * 128:(tb + 1) * 128])
                            nc.vector.tensor_copy(dst, pt[:])
                        nc.vector.tensor_copy(v_sb[:, tb, :], pq[:, 384:512])

                _ckpt(f"qkv{b}")
                # ---- banded attention, query-block pairs ----
                for p in range(4):
                    ukb0 = max(0, 2 * p - 4)
                    nu = 2 * p + 2 - ukb0
                    keys = nu * 128
                    enc = ap_["encp"].tile([128, 2, 256], F32R)
                    for h in range(2):
                        probsT = ap_["pT"].tile([128, 6, 256], F32R)
                        for qi in range(2):
                            qb = 2 * p + qi
                            psl = psL.tile([128, 768], F32)
                            c0 = 0
                            while c0 < keys:
                                c1 = min(c0 + 512, keys)
                                nc.tensor.matmul(
                                    psl[:, c0:c1],
                                    qT[:, h, qb * 128:(qb + 1) * 128],
                                    kT[:, ukb0 * 128 + c0: ukb0 * 128 + c1],
                                    start=True, stop=True)
                                c0 = c1
                            t1 = ap_["t1"].tile([128, 768], F32)
                            nc.scalar.activation(t1[:, 0:keys], psl[:, 0:keys],
                                                 AF.Tanh, scale=1.0 / CAP)
                            for j in range(nu):
                                mi = _midx(qb, ukb0 + j)
                                nc.vector.scalar_tensor_tensor(
                                    out=t1[:, j * 128:(j + 1) * 128],
                                    in0=t1[:, j * 128:(j + 1) * 128],
                                    scalar=CAP, in1=maskb_sb[:, mi, :],
                                    op0=OP.mult, op1=OP.add)
                            ex = ap_["exp"].tile([128, 768], F32)
                            den = small.tile([128, 1], F32)
                            nc.scalar.activation(ex[:, 0:keys], t1[:, 0:keys],
                                                 AF.Exp, accum_out=den[:])
                            nc.vector.reciprocal(den[:], den[:])
                            prob = ap_["prob"].tile([128, 768], F32R)
                            nc.vector.tensor_scalar_mul(
                                prob[:, 0:keys], ex[:, 0:keys], den[:])
                            for j in range(nu):
                                pt = psT.tile([128, 128], F32R)
                                nc.tensor.transpose(
                                    pt[:], prob[:, j * 128:(j + 1) * 128],
                                    iden_sb[:])
                                nc.vector.tensor_copy(
                                    probsT[:, j, qi * 128:(qi + 1) * 128],
                                    pt[:])
                        pav = psA.tile([128, 256], F32, tag="mm")
                        for j in range(nu):
                            nc.tensor.matmul(
                                pav[:], v_sb[:, ukb0 + j, :], probsT[:, j, :],
                                start=(j == 0), stop=(j == nu - 1))
                        nc.vector.tensor_copy(enc[:, h, :], pav[:])
                    # o-projection (partial over this core's 2 heads)
                    for qi in range(2):
                        qb = 2 * p + qi
                        for ch in range(4):
                            po = psA.tile([128, 512], F32, tag="mm")
                            for h in range(2):
                                nc.tensor.matmul(
                                    po[:], enc[:, h, qi * 128:(qi + 1) * 128],
                                    ow_sb[:, h, ch, :],
                                    start=(h == 0), stop=(h == 1))
                            ob = obp.tile([128, 512], F32, tag="ob", name="ob")
                            nc.vector.tensor_copy(ob[:], po[:])
                            nc.sync.dma_start(
                                o_part[b * T + qb * 128: b * T + (qb + 1) * 128,
                                       ch * 512:(ch + 1) * 512], ob[:])

                _ckpt(f"oproj{b}")
                _rscatter(o_part[b * T:(b + 1) * T, :],
                          rs1[b * 128:(b + 1) * 128, :], fake_rs1)
                _bchain(b)
                _ckpt(f"bchain{b}")

        if not do_attn:
            for b in range(B):
                nc.sync.dma_start(rs1[b * 128:(b + 1) * 128, :],
                                  x[b * T: b * T + 128, :])
                _bchain(b)

        # =================== MLP (TP over hidden dim) ===================
        with ExitStack() as mlp_scope:
          if not do_mlp:
            for b in range(B):
                nc.sync.dma_start(rs2[b * 128:(b + 1) * 128, :],
                                  rs1[b * 128:(b + 1) * 128, :])
          else:
            mp_ = {}
            for nm, args in [
                ("h2Tp", dict(bufs=1)), ("wst", dict(bufs=2)),
                ("actp", dict(bufs=1)), ("dwp", dict(bufs=1)),
            ]:
                mp_[nm] = mlp_scope.enter_context(tc.tile_pool(name=nm, **args))
            psM = mlp_scope.enter_context(
                tc.tile_pool(name="psM", bufs=TUNE["psM"], space="PSUM"))

            for b in range(B):
                h2Tf = mp_["h2Tp"].tile([128, DT, 1024], F32R)
                for r in range(NCORES):
                    rv = ag_outs[b][r * D:(r + 1) * D, :].rearrange(
                        "(dt p) c -> p dt c", p=128)
                    for g in range(2):
                        nc.sync.dma_start(
                            h2Tf[:, g * 8:(g + 1) * 8, r * 128:(r + 1) * 128],
                            rv[:, g * 8:(g + 1) * 8, :])
                actT = mp_["actp"].tile([128, 8, 1024], F32R)
                for hc in range(8):
                    gw_t = mp_["wst"].tile([128, DT, 128], F32R, tag="w")
                    gv = gw[:, hc * 128:(hc + 1) * 128].rearrange(
                        "(dt p) h -> p dt h", p=128)
                    uw_t = mp_["wst"].tile([128, DT, 128], F32R, tag="w")
                    uv = uw[:, hc * 128:(hc + 1) * 128].rearrange(
                        "(dt p) h -> p dt h", p=128)
                    for g in range(4):
                        nc.sync.dma_start(gw_t[:, g * 4:(g + 1) * 4, :],
                                          gv[:, g * 4:(g + 1) * 4, :])
                        nc.sync.dma_start(uw_t[:, g * 4:(g + 1) * 4, :],
                                          uv[:, g * 4:(g + 1) * 4, :])
                    for tch in range(2):
                        psg = psM.tile([128, 512], F32, tag="psm")
                        psu = psM.tile([128, 512], F32, tag="psm")
                        for dt in range(DT):
                            nc.tensor.matmul(
                                psg[:], gw_t[:, dt, :],
                                h2Tf[:, dt, tch * 512:(tch + 1) * 512],
                                start=(dt == 0), stop=(dt == DT - 1))
                        for dt in range(DT):
                            nc.tensor.matmul(
                                psu[:], uw_t[:, dt, :],
                                h2Tf[:, dt, tch * 512:(tch + 1) * 512],
                                start=(dt == 0), stop=(dt == DT - 1))
                        gel = obp.tile([128, 512], F32, tag="ob", name="gel")
                        nc.scalar.activation(gel[:], psg[:],
                                             AF.Gelu_apprx_tanh)
                        nc.vector.tensor_mul(
                            actT[:, hc, tch * 512:(tch + 1) * 512],
                            gel[:], psu[:])
                _ckpt(f"gateup{b}")
                # down-projection, streamed per 512-wide D chunk
                for ch in range(4):
                    dw_t = mp_["dwp"].tile([128, 8, 512], F32R)
                    for hc in range(8):
                        nc.sync.dma_start(
                            dw_t[:, hc, :],
                            dw[hc * 128:(hc + 1) * 128,
                               ch * 512:(ch + 1) * 512])
                    for tb in range(TB):
                        psd = psM.tile([128, 512], F32, tag="psm")
                        for hc in range(8):
                            nc.tensor.matmul(
                                psd[:], actT[:, hc, tb * 128:(tb + 1) * 128],
                                dw_t[:, hc, :],
                                start=(hc == 0), stop=(hc == 7))
                        mb = obp.tile([128, 512], F32, tag="ob", name="mb")
                        nc.vector.tensor_copy(mb[:], psd[:])
                        nc.sync.dma_start(
                            mlp_part[b * T + tb * 128: b * T + (tb + 1) * 128,
                                     ch * 512:(ch + 1) * 512], mb[:])
                _ckpt(f"down{b}")
                _rscatter(mlp_part[b * T:(b + 1) * T, :],
                          rs2[b * 128:(b + 1) * 128, :], fake_rs2)
                _ckpt(f"rs2_{b}")

        # =================== final norm + residual ===================
        postffw_bc = bcast.tile([128, D], F32, tag="bc")
        _bcast_row(nc, postffw_bc[:], postffwmul)
        for b in range(B):
            rst = bwork.tile([128, D], F32, tag="bw")
            _dma4(nc, rst[:], rs2[b * 128:(b + 1) * 128, :])
            tmp = bwork.tile([128, D], F32, tag="bw")
            _rms(nc, pools, rst[:], tmp[:], postffw_bc[:], D, tmp[:])
            aol = bwork.tile([128, D], F32, tag="bw")
            _dma4(nc, aol[:], attn_out_d[b * 128:(b + 1) * 128, :])
            nc.vector.tensor_add(tmp[:], tmp[:], aol[:])
            _dma4(nc, out[b * 128:(b + 1) * 128, :], tmp[:])


# ---------------------------------------------------------------------------
# host side
# ---------------------------------------------------------------------------

_NC = None


def _get_nc():
    global _NC
    if _NC is None:
        _NC = _build_program()
    return _NC


def _host_prep(inputs):
    """Build the 8 per-core input maps from the full problem inputs."""
    x = np.ascontiguousarray(np.asarray(inputs["x"], dtype=np.float32))
    seg = np.asarray(inputs["segment_pos"], dtype=np.int32)
    am = np.asarray(inputs["attn_mask"])
    q_k = np.asarray(inputs["q_kernel"], dtype=np.float32)
    kv_k = np.asarray(inputs["kv_kernel"], dtype=np.float32)
    o_k = np.asarray(inputs["o_kernel"], dtype=np.float32)
    gate_w = np.asarray(inputs["gate_w"], dtype=np.float32)
    up_w = np.asarray(inputs["up_w"], dtype=np.float32)
    down_w = np.asarray(inputs["down_w"], dtype=np.float32)

    xf = x.reshape(B * T, D)
    premul = (1.0 + np.asarray(inputs["pre_attn_scale"], np.float32))
    postattn = (1.0 + np.asarray(inputs["post_attn_scale"], np.float32))
    preffw = (1.0 + np.asarray(inputs["pre_ffw_scale"], np.float32))
    postffw = (1.0 + np.asarray(inputs["post_ffw_scale"], np.float32))
    qmul = ((1.0 + np.asarray(inputs["q_norm_scale"], np.float32))
            * np.float32(H ** -0.5)).astype(np.float32)
    kmul = (1.0 + np.asarray(inputs["k_norm_scale"], np.float32))

    frac = (2.0 * np.arange(H // 2, dtype=np.float32) / H).astype(np.float32)
    ts = (ROPE_BASE ** frac).astype(np.float32)
    sinu = (seg[..., None].astype(np.float32) / ts).astype(np.float32)
    cosb = np.cos(sinu).reshape(B * T, 64).astype(np.float32)
    sinb = np.sin(sinu).reshape(B * T, 64).astype(np.float32)

    ti = np.arange(128)[:, None]
    si = np.arange(128)[None, :]
    maskb = np.stack([
        np.full((128, 128), KMASK, np.float32),
        np.where(ti >= si, 0.0, KMASK).astype(np.float32),
        np.zeros((128, 128), np.float32),
        np.where(ti < si, 0.0, KMASK).astype(np.float32),
    ]).astype(np.float32)

    # soft structural check of the actual mask against the canonical tiles
    tt = np.arange(T)
    sliding = (np.abs(tt[:, None] - tt[None, :]) <= WINDOW - 1)
    expected = am & sliding[None]
    ok = True
    for qb in range(min(2, TB)):
        for kb in range(qb + 1):
            blk = np.where(expected[0, qb * 128:(qb + 1) * 128,
                                    kb * 128:(kb + 1) * 128], 0.0, KMASK)
            if not np.array_equal(blk.astype(np.float32),
                                  maskb[_midx(qb, kb)]):
                ok = False
    if not ok:
        print("kernel.py WARNING: attn_mask does not match canonical "
              "causal+sliding structure; results may be wrong")

    iden = np.eye(128, dtype=np.float32)

    in_maps = []
    for c in range(NCORES):
        qw_c = q_k[2 * c:2 * c + 2].transpose(1, 0, 2).reshape(D, 256)
        kw_c = kv_k[0, c]
        vw_c = kv_k[1, c]
        wqkv_c = np.ascontiguousarray(
            np.concatenate([qw_c, kw_c, vw_c], axis=1), dtype=np.float32)
        ow_c = np.ascontiguousarray(
            o_k[2 * c:2 * c + 2].reshape(256, D), dtype=np.float32)
        gw_c = np.ascontiguousarray(gate_w[:, 1024 * c:1024 * (c + 1)])
        uw_c = np.ascontiguousarray(up_w[:, 1024 * c:1024 * (c + 1)])
        dw_c = np.ascontiguousarray(down_w[1024 * c:1024 * (c + 1), :])
        xsh_c = np.ascontiguousarray(np.concatenate(
            [xf[128 * c:128 * (c + 1)],
             xf[T + 128 * c: T + 128 * (c + 1)]], axis=0))
        in_maps.append({
            "x": xf, "xsh": xsh_c, "wqkv": wqkv_c, "ow": ow_c,
            "gw": gw_c, "uw": uw_c, "dw": dw_c,
            "cosb": cosb, "sinb": sinb, "maskb": maskb,
            "premul": premul, "postattnmul": postattn,
            "preffwmul": preffw, "postffwmul": postffw,
            "qmul": qmul, "kmul": kmul, "iden": iden,
        })
    return in_maps


def _assemble(results):
    out = np.empty((B, T, D), dtype=np.float32)
    for c in range(NCORES):
        r = results[c]["out"]
        out[0, 128 * c:128 * (c + 1)] = r[0:128]
        out[1, 128 * c:128 * (c + 1)] = r[128:256]
    return out


def kernel(**inputs) -> np.ndarray:
    from concourse import bass_utils
    nc = _get_nc()
    in_maps = _host_prep(inputs)
    r = bass_utils.run_bass_kernel_spmd(nc, in_maps,
                                        core_ids=list(range(NCORES)))
    return _assemble(r.results)


# revision 61
# speedup vs baseline: 1.4477x; 1.1019x over previous
"""Gemma-style transformer block (GQA + sliding-window attention + gated-GELU
MLP) on 8 Trainium2 NeuronCores.

Sharding (Megatron + sequence-parallel):
  - Attention: tensor-parallel over heads. Core c owns q heads {2c, 2c+1} and
    kv head c. Each core computes full-sequence attention for its heads plus
    its partial o-projection; a per-batch ReduceScatter (over tokens) completes
    the sum, leaving core c with tokens [128c, 128c+128) of each batch.
  - Norms + residuals run on the token shard (sequence-parallel).
  - MLP: tensor-parallel over the hidden dim (core c owns HID slice
    [1024c, 1024c+1024)). An AllGather of the (transposed) h2 shard feeds the
    gate/up matmuls; a second per-batch ReduceScatter completes down-proj.
  - Host concatenates the 8 per-core [256, D] output shards.

Matmuls run as float32r (full-rate fp32 variant, ~1.6e-4 component error);
everything else is fp32.
"""
import sys

sys.path.insert(0, "/opt/trn_rl_repo")

import numpy as np

import concourse.bass as bass
import concourse.mybir as mybir
import concourse.tile as tile
from concourse import bacc

F32 = mybir.dt.float32
F32R = mybir.dt.float32r
AF = mybir.ActivationFunctionType
OP = mybir.AluOpType

B, T, D = 2, 1024, 2048
NQ, KV, H, HID = 16, 8, 128, 8192
WINDOW, CAP = 512, 50.0
KMASK = -2.3819763e38
EPS = 1e-6
ROPE_BASE = 10000.0
NCORES = 8
DT = D // 128          # 16 contraction tiles over D
TB = T // 128          # 8 token blocks per batch
RG = [list(range(NCORES))]

TUNE = {"psA": 2, "psL": 2, "psT": 2, "pT": 2, "hT": 3, "prob": 1,
        "psM": 3, "encp": 2}


def _midx(qb, kb):
    """Canonical additive-mask tile index for (query block, key block)."""
    if kb > qb:
        return 0       # future block: fully masked
    if kb == qb:
        return 1       # causal lower-tri (incl diag)
    if kb == qb - 4:
        return 3       # window tail: strict upper-tri allowed
    if kb < qb - 4:
        return 0       # fully outside window
    return 2           # fully inside window: no mask


def _rms(nc, pools, in_ap, out_ap, mul_bc, width, sq_tile):
    """out = in * rsqrt(mean(in^2)+EPS) * mul   (mul broadcast tile)."""
    ss = pools["small"].tile([128, 1], F32)
    nc.scalar.activation(sq_tile, in_ap, AF.Square, accum_out=ss[:])
    rs = pools["small"].tile([128, 1], F32)
    nc.scalar.activation(rs[:], ss[:], AF.Sqrt, scale=1.0 / width,
                         bias=pools["eps"][:])
    nc.vector.reciprocal(rs[:], rs[:])
    nc.vector.scalar_tensor_tensor(out=out_ap, in0=in_ap, scalar=rs[:],
                                   in1=mul_bc, op0=OP.mult, op1=OP.mult)


def _dma4(nc, dst, src, n=4):
    """Split a [128, W] (or matching) DMA into n column chunks across queues."""
    W = dst.shape[-1]
    step = W // n
    for i in range(n):
        sl = (slice(None),) * (len(dst.shape) - 1)
        nc.sync.dma_start(dst[(*sl, slice(i * step, (i + 1) * step))],
                          src[(*(slice(None),) * (len(src.shape) - 1),
                               slice(i * step, (i + 1) * step))])


def _bcast_row(nc, dst, src_ap):
    """DMA a [W] dram vector broadcast to a [P, W] sbuf tile."""
    nc.sync.dma_start(dst, bass.AP(
        tensor=src_ap.tensor, offset=src_ap.offset,
        ap=[[0, dst.shape[0]], *src_ap.ap]))


def _build_program(reps=1, single=False, do_attn=True, do_mlp=True,
                   fake_coll=False, stop_after=None, fake_rs1=False,
                   fake_ag=False, fake_rs2=False):
    nc = bacc.Bacc("TRN2", target_bir_lowering=False, debug=False,
                   enable_asserts=True,
                   num_devices=(1 if single else NCORES))

    def din(name, shape, dt=F32):
        return nc.dram_tensor(name, shape, dt, kind="ExternalInput").ap()

    x = din("x", [B * T, D])
    xsh = din("xsh", [2 * 128, D])
    wqkv = din("wqkv", [D, 512], F32R)          # [D, 2H q | H k | H v]
    ow = din("ow", [256, D], F32R)              # [2*H rows, D]
    gw = din("gw", [D, 1024], F32R)
    uw = din("uw", [D, 1024], F32R)
    dw = din("dw", [1024, D], F32R)
    cosb = din("cosb", [B * T, 64])
    sinb = din("sinb", [B * T, 64])
    maskb = din("maskb", [4, 128, 128])
    premul = din("premul", [D])
    postattnmul = din("postattnmul", [D])
    preffwmul = din("preffwmul", [D])
    postffwmul = din("postffwmul", [D])
    qmul = din("qmul", [H])
    kmul = din("kmul", [H])
    iden = din("iden", [128, 128], F32R)

    out = nc.dram_tensor("out", [2 * 128, D], F32, kind="ExternalOutput").ap()

    with tile.TileContext(nc) as tc:
        for _ in range(reps):
            _body(nc, tc, x=x, xsh=xsh, wqkv=wqkv, ow=ow, gw=gw, uw=uw,
                  dw=dw, cosb=cosb, sinb=sinb, maskb=maskb, premul=premul,
                  postattnmul=postattnmul, preffwmul=preffwmul,
                  postffwmul=postffwmul, qmul=qmul, kmul=kmul, iden=iden,
                  out=out, single=single, do_attn=do_attn, do_mlp=do_mlp,
                  fake_coll=fake_coll, stop_after=stop_after,
                  fake_rs1=fake_rs1, fake_ag=fake_ag, fake_rs2=fake_rs2)
    nc.compile()
    return nc


def _body(nc, tc, *, x, xsh, wqkv, ow, gw, uw, dw, cosb, sinb, maskb, premul,
          postattnmul, preffwmul, postffwmul, qmul, kmul, iden, out,
          single=False, do_attn=True, do_mlp=True, fake_coll=False,
          stop_after=None, fake_rs1=False, fake_ag=False, fake_rs2=False):
    from contextlib import ExitStack

    if single:
        fake_coll = True
    if fake_coll:
        fake_rs1 = fake_ag = fake_rs2 = True

    class _Stop(Exception):
        pass

    def _ckpt(name):
        if stop_after == name:
            raise _Stop()

    def _rscatter(in_ap, out_ap, fake):
        if fake:
            nrows = out_ap.shape[0]
            nc.sync.dma_start(out_ap, in_ap[0:nrows, :])
        else:
            nc.gpsimd.collective_compute(
                "ReduceScatter", OP.add, replica_groups=RG,
                ins=[in_ap.opt()], outs=[out_ap.opt()])

    def _agather(in_ap, out_ap):
        if fake_ag:
            nrows = in_ap.shape[0]
            for r in range(NCORES):
                nc.sync.dma_start(out_ap[r * nrows:(r + 1) * nrows, :], in_ap)
        else:
            nc.gpsimd.collective_compute(
                "AllGather", OP.bypass, replica_groups=RG,
                ins=[in_ap.opt()], outs=[out_ap.opt()])

    try:
        _body_inner(nc, tc, _Stop=_Stop, _ckpt=_ckpt, _rscatter=_rscatter,
                    _agather=_agather, x=x, xsh=xsh, wqkv=wqkv, ow=ow, gw=gw,
                    uw=uw, dw=dw, cosb=cosb, sinb=sinb, maskb=maskb,
                    premul=premul, postattnmul=postattnmul,
                    preffwmul=preffwmul, postffwmul=postffwmul, qmul=qmul,
                    kmul=kmul, iden=iden, out=out, single=single,
                    do_attn=do_attn, do_mlp=do_mlp, fake_coll=fake_coll,
                    fake_rs1=fake_rs1, fake_ag=fake_ag, fake_rs2=fake_rs2)
    except _Stop:
        pass


def _body_inner(nc, tc, *, _Stop, _ckpt, _rscatter, _agather, x, xsh, wqkv,
                ow, gw, uw, dw, cosb, sinb, maskb, premul, postattnmul,
                preffwmul, postffwmul, qmul, kmul, iden, out, single, do_attn,
                do_mlp, fake_coll, fake_rs1=False, fake_ag=False,
                fake_rs2=False):
    from contextlib import ExitStack

    est = ExitStack()
    with est:
        # ----- long-lived pools -----
        consts = est.enter_context(tc.tile_pool(name="consts", bufs=1))
        bcast = est.enter_context(tc.tile_pool(name="bcast", bufs=3))
        small = est.enter_context(tc.tile_pool(name="small", bufs=8))
        obp = est.enter_context(tc.tile_pool(name="obp", bufs=3))
        bwork = est.enter_context(tc.tile_pool(name="bwork", bufs=5))
        dram = est.enter_context(tc.tile_pool(name="dram", bufs=1,
                                              space="DRAM"))
        psA = est.enter_context(tc.tile_pool(name="psA", bufs=TUNE["psA"], space="PSUM"))
        psT = est.enter_context(tc.tile_pool(name="psT", bufs=TUNE["psT"], space="PSUM"))

        iden_sb = consts.tile([128, 128], F32R)
        nc.sync.dma_start(iden_sb[:], iden[:])
        qmul_bc = consts.tile([128, H], F32)
        _bcast_row(nc, qmul_bc[:], qmul)
        kmul_bc = consts.tile([128, H], F32)
        _bcast_row(nc, kmul_bc[:], kmul)
        eps_t = consts.tile([128, 1], F32)
        nc.vector.memset(eps_t[:], EPS)
        pools = {"small": small, "eps": eps_t}

        premul_bc = bcast.tile([128, D], F32, tag="bc")
        _bcast_row(nc, premul_bc[:], premul)
        postattn_bc = bcast.tile([128, D], F32, tag="bc")
        _bcast_row(nc, postattn_bc[:], postattnmul)
        preffw_bc = bcast.tile([128, D], F32, tag="bc")
        _bcast_row(nc, preffw_bc[:], preffwmul)

        # DRAM intermediates
        attn_out_d = dram.tile([B * 128, D], F32)
        o_part = dram.tile([B * T, D], F32)
        rs1 = dram.tile([B * 128, D], F32)
        ag_in = dram.tile([B * D, 128], F32R)
        ag_sp = "Local" if fake_ag else "Shared"
        ag_outs = [
            dram.tile([NCORES * D, 128], F32R, addr_space=ag_sp,
                      tag=f"ag_out{b}", name=f"ag_out{b}")
            for b in range(B)
        ]
        mlp_chs = [dram.tile([B * T, 512], F32, tag=f"mlpc{c}",
                             name=f"mlpc{c}") for c in range(4)]
        rs2_chs = [dram.tile([B * 128, 512], F32, tag=f"rs2c{c}",
                             name=f"rs2c{c}") for c in range(4)]

        def _bchain(b):
            """post-attn norm + residual + pre-ffw norm + h2T + AllGather,
            on this core's token shard of batch b (sequence-parallel)."""
            rst = bwork.tile([128, D], F32, tag="bw", name="rst")
            _dma4(nc, rst[:], rs1[b * 128:(b + 1) * 128, :])
            ao = bwork.tile([128, D], F32, tag="bw", name="ao")
            _rms(nc, pools, rst[:], ao[:], postattn_bc[:], D, ao[:])
            xst = bwork.tile([128, D], F32, tag="bw", name="xst")
            _dma4(nc, xst[:], xsh[b * 128:(b + 1) * 128, :])
            nc.vector.tensor_add(ao[:], ao[:], xst[:])
            _dma4(nc, attn_out_d[b * 128:(b + 1) * 128, :], ao[:])
            h2 = bwork.tile([128, D], F32R, tag="bw", name="h2")
            sq2 = bwork.tile([128, D], F32, tag="bw", name="sq2")
            _rms(nc, pools, ao[:], h2[:], preffw_bc[:], D, sq2[:])
            h2T = bwork.tile([128, DT, 128], F32R, tag="bw", name="h2T")
            for dt in range(DT):
                pt = psT.tile([128, 128], F32R)
                nc.tensor.transpose(pt[:], h2[:, dt * 128:(dt + 1) * 128],
                                    iden_sb[:])
                nc.vector.tensor_copy(h2T[:, dt, :], pt[:])
            agv = ag_in[b * D:(b + 1) * D, :].rearrange(
                "(dt p) c -> p dt c", p=128)
            for g in range(4):
                nc.sync.dma_start(agv[:, g * 4:(g + 1) * 4, :],
                                  h2T[:, g * 4:(g + 1) * 4, :])
            _agather(ag_in[b * D:(b + 1) * D, :], ag_outs[b][:, :])

        # =================== ATTENTION (TP over heads) ===================
        with ExitStack() as attn_scope:
          if do_attn:
            ap_ = {}
            for nm, args in [
                ("csin", dict(bufs=2)), ("maskp", dict(bufs=1)),
                ("wqkvp", dict(bufs=1)), ("owp", dict(bufs=1)),
                ("hT", dict(bufs=TUNE["hT"])), ("hp", dict(bufs=1)),
                ("sqs", dict(bufs=1)), ("nrm", dict(bufs=2)),
                ("ro", dict(bufs=2)), ("t64", dict(bufs=2)),
                ("qT", dict(bufs=1)), ("kT", dict(bufs=1)),
                ("vp", dict(bufs=1)), ("t1", dict(bufs=1)),
                ("exp", dict(bufs=1)), ("prob", dict(bufs=TUNE["prob"])),
                ("pT", dict(bufs=TUNE["pT"])), ("encp", dict(bufs=TUNE["encp"])),
                ("bh2", dict(bufs=1)),
            ]:
                ap_[nm] = attn_scope.enter_context(
                    tc.tile_pool(name=nm, **args))
            psL = attn_scope.enter_context(
                tc.tile_pool(name="psL", bufs=TUNE["psL"], space="PSUM"))

            maskb_sb = ap_["maskp"].tile([128, 4, 128], F32)
            nc.sync.dma_start(maskb_sb[:], maskb.rearrange("m p k -> p m k"))
            wqkv_sb = ap_["wqkvp"].tile([128, DT, 512], F32R)
            for dt in range(DT):
                nc.sync.dma_start(wqkv_sb[:, dt, :],
                                  wqkv[dt * 128:(dt + 1) * 128, :])
            ow_sb = ap_["owp"].tile([128, 2, 4, 512], F32R)
            for hh in range(2):
                for ch in range(4):
                    nc.sync.dma_start(
                        ow_sb[:, hh, ch, :],
                        ow[hh * 128:(hh + 1) * 128, ch * 512:(ch + 1) * 512])

            for b in range(B):
                cos_t = ap_["csin"].tile([128, TB, 64], F32, tag="cs")
                nc.sync.dma_start(
                    cos_t[:], cosb[b * T:(b + 1) * T, :].rearrange(
                        "(tb p) h -> p tb h", p=128))
                sin_t = ap_["csin"].tile([128, TB, 64], F32, tag="cs")
                nc.sync.dma_start(
                    sin_t[:], sinb[b * T:(b + 1) * T, :].rearrange(
                        "(tb p) h -> p tb h", p=128))

                qT = ap_["qT"].tile([128, 2, T], F32R)
                kT = ap_["kT"].tile([128, T], F32R)
                v_sb = ap_["vp"].tile([128, TB, 128], F32R)

                for tb in range(TB):
                    hT = ap_["hT"].tile([128, DT, 128], F32R)
                    # ---- h = rms(x)*premul, transposed into hT ----
                    if True:
                        xt = bwork.tile([128, D], F32, tag="bw")
                        _dma4(nc, xt[:],
                              x[b * T + tb * 128: b * T + (tb + 1) * 128, :])
                        ht = ap_["hp"].tile([128, D], F32R)
                        sq = bwork.tile([128, D], F32, tag="bw")
                        _rms(nc, pools, xt[:], ht[:], premul_bc[:], D, sq[:])
                        for dt in range(DT):
                            pt = psT.tile([128, 128], F32R)
                            nc.tensor.transpose(
                                pt[:], ht[:, dt * 128:(dt + 1) * 128],
                                iden_sb[:])
                            nc.vector.tensor_copy(hT[:, dt, :], pt[:])
                    # ---- qkv projections + qk-norm + rope ----
                    if True:
                        pq = psA.tile([128, 512], F32, tag="mm")
                        for dt in range(DT):
                            nc.tensor.matmul(
                                pq[:], hT[:, dt, :], wqkv_sb[:, dt, :],
                                start=(dt == 0), stop=(dt == DT - 1))
                        for hd in range(3):    # q0, q1, k
                            sl = pq[:, hd * 128:(hd + 1) * 128]
                            nrm = ap_["nrm"].tile([128, 128], F32)
                            sqs = ap_["sqs"].tile([128, 128], F32)
                            _rms(nc, pools, sl, nrm[:],
                                 (qmul_bc if hd < 2 else kmul_bc)[:], H,
                                 sqs[:])
                            ro = ap_["ro"].tile([128, 128], F32R)
                            ct, st = cos_t[:, tb, :], sin_t[:, tb, :]
                            t1 = ap_["t64"].tile([128, 64], F32)
                            t2 = ap_["t64"].tile([128, 64], F32)
                            nc.vector.tensor_mul(t1[:], nrm[:, 0:64], ct)
                            nc.vector.tensor_mul(t2[:], nrm[:, 64:128], st)
                            nc.vector.tensor_sub(ro[:, 0:64], t1[:], t2[:])
                            t3 = ap_["t64"].tile([128, 64], F32)
                            t4_ = ap_["t64"].tile([128, 64], F32)
                            nc.vector.tensor_mul(t3[:], nrm[:, 64:128], ct)
                            nc.vector.tensor_mul(t4_[:], nrm[:, 0:64], st)
                            nc.vector.tensor_add(ro[:, 64:128], t3[:], t4_[:])
                            pt = psT.tile([128, 128], F32R)
                            nc.tensor.transpose(pt[:], ro[:], iden_sb[:])
                            dst = (qT[:, hd, tb * 128:(tb + 1) * 128]
                                   if hd < 2 else
                                   kT[:, tb * 128:(tb + 1) * 128])
                            nc.vector.tensor_copy(dst, pt[:])
                        nc.vector.tensor_copy(v_sb[:, tb, :], pq[:, 384:512])

                _ckpt(f"qkv{b}")
                # ---- banded attention, query-block pairs ----
                for p in range(4):
                    ukb0 = max(0, 2 * p - 4)
                    nu = 2 * p + 2 - ukb0
                    keys = nu * 128
                    enc = ap_["encp"].tile([128, 2, 256], F32R)
                    for h in range(2):
                        probsT = ap_["pT"].tile([128, 6, 256], F32R)
                        for qi in range(2):
                            qb = 2 * p + qi
                            psl = psL.tile([128, 768], F32)
                            c0 = 0
                            while c0 < keys:
                                c1 = min(c0 + 512, keys)
                                nc.tensor.matmul(
                                    psl[:, c0:c1],
                                    qT[:, h, qb * 128:(qb + 1) * 128],
                                    kT[:, ukb0 * 128 + c0: ukb0 * 128 + c1],
                                    start=True, stop=True)
                                c0 = c1
                            t1 = ap_["t1"].tile([128, 768], F32)
                            nc.scalar.activation(t1[:, 0:keys], psl[:, 0:keys],
                                                 AF.Tanh, scale=1.0 / CAP)
                            for j in range(nu):
                                mi = _midx(qb, ukb0 + j)
                                nc.vector.scalar_tensor_tensor(
                                    out=t1[:, j * 128:(j + 1) * 128],
                                    in0=t1[:, j * 128:(j + 1) * 128],
                                    scalar=CAP, in1=maskb_sb[:, mi, :],
                                    op0=OP.mult, op1=OP.add)
                            ex = ap_["exp"].tile([128, 768], F32)
                            den = small.tile([128, 1], F32)
                            nc.scalar.activation(ex[:, 0:keys], t1[:, 0:keys],
                                                 AF.Exp, accum_out=den[:])
                            nc.vector.reciprocal(den[:], den[:])
                            prob = ap_["prob"].tile([128, 768], F32R)
                            nc.vector.tensor_scalar_mul(
                                prob[:, 0:keys], ex[:, 0:keys], den[:])
                            for j in range(nu):
                                pt = psT.tile([128, 128], F32R)
                                nc.tensor.transpose(
                                    pt[:], prob[:, j * 128:(j + 1) * 128],
                                    iden_sb[:])
                                nc.vector.tensor_copy(
                                    probsT[:, j, qi * 128:(qi + 1) * 128],
                                    pt[:])
                        pav = psA.tile([128, 256], F32, tag="mm")
                        for j in range(nu):
                            nc.tensor.matmul(
                                pav[:], v_sb[:, ukb0 + j, :], probsT[:, j, :],
                                start=(j == 0), stop=(j == nu - 1))
                        nc.vector.tensor_copy(enc[:, h, :], pav[:])
                    # o-projection (partial over this core's 2 heads)
                    for qi in range(2):
                        qb = 2 * p + qi
                        for ch in range(4):
                            po = psA.tile([128, 512], F32, tag="mm")
                            for h in range(2):
                                nc.tensor.matmul(
                                    po[:], enc[:, h, qi * 128:(qi + 1) * 128],
                                    ow_sb[:, h, ch, :],
                                    start=(h == 0), stop=(h == 1))
                            ob = obp.tile([128, 512], F32, tag="ob", name="ob")
                            nc.vector.tensor_copy(ob[:], po[:])
                            nc.sync.dma_start(
                                o_part[b * T + qb * 128: b * T + (qb + 1) * 128,
                                       ch * 512:(ch + 1) * 512], ob[:])

                _ckpt(f"oproj{b}")
                _rscatter(o_part[b * T:(b + 1) * T, :],
                          rs1[b * 128:(b + 1) * 128, :], fake_rs1)
                _bchain(b)
                _ckpt(f"bchain{b}")

        if not do_attn:
            for b in range(B):
                nc.sync.dma_start(rs1[b * 128:(b + 1) * 128, :],
                                  x[b * T: b * T + 128, :])
                _bchain(b)

        # =================== MLP (TP over hidden dim) ===================
        with ExitStack() as mlp_scope:
          if not do_mlp:
            for b in range(B):
                nc.sync.dma_start(rs2[b * 128:(b + 1) * 128, :],
                                  rs1[b * 128:(b + 1) * 128, :])
          else:
            mp_ = {}
            for nm, args in [
                ("h2Tp", dict(bufs=1)), ("wst", dict(bufs=2)),
                ("actp", dict(bufs=1)), ("dwp", dict(bufs=1)),
            ]:
                mp_[nm] = mlp_scope.enter_context(tc.tile_pool(name=nm, **args))
            psM = mlp_scope.enter_context(
                tc.tile_pool(name="psM", bufs=TUNE["psM"], space="PSUM"))

            for b in range(B):
                h2Tf = mp_["h2Tp"].tile([128, DT, 1024], F32R)
                for r in range(NCORES):
                    rv = ag_outs[b][r * D:(r + 1) * D, :].rearrange(
                        "(dt p) c -> p dt c", p=128)
                    for g in range(2):
                        nc.sync.dma_start(
                            h2Tf[:, g * 8:(g + 1) * 8, r * 128:(r + 1) * 128],
                            rv[:, g * 8:(g + 1) * 8, :])
                actT = mp_["actp"].tile([128, 8, 1024], F32R)
                for hc in range(8):
                    gw_t = mp_["wst"].tile([128, DT, 128], F32R, tag="w")
                    gv = gw[:, hc * 128:(hc + 1) * 128].rearrange(
                        "(dt p) h -> p dt h", p=128)
                    uw_t = mp_["wst"].tile([128, DT, 128], F32R, tag="w")
                    uv = uw[:, hc * 128:(hc + 1) * 128].rearrange(
                        "(dt p) h -> p dt h", p=128)
                    for g in range(4):
                        nc.sync.dma_start(gw_t[:, g * 4:(g + 1) * 4, :],
                                          gv[:, g * 4:(g + 1) * 4, :])
                        nc.sync.dma_start(uw_t[:, g * 4:(g + 1) * 4, :],
                                          uv[:, g * 4:(g + 1) * 4, :])
                    for tch in range(2):
                        psg = psM.tile([128, 512], F32, tag="psm")
                        psu = psM.tile([128, 512], F32, tag="psm")
                        for dt in range(DT):
                            nc.tensor.matmul(
                                psg[:], gw_t[:, dt, :],
                                h2Tf[:, dt, tch * 512:(tch + 1) * 512],
                                start=(dt == 0), stop=(dt == DT - 1))
                        for dt in range(DT):
                            nc.tensor.matmul(
                                psu[:], uw_t[:, dt, :],
                                h2Tf[:, dt, tch * 512:(tch + 1) * 512],
                                start=(dt == 0), stop=(dt == DT - 1))
                        gel = obp.tile([128, 512], F32, tag="ob", name="gel")
                        nc.scalar.activation(gel[:], psg[:],
                                             AF.Gelu_apprx_tanh)
                        nc.vector.tensor_mul(
                            actT[:, hc, tch * 512:(tch + 1) * 512],
                            gel[:], psu[:])
                _ckpt(f"gateup{b}")
                # down-projection, streamed per 512-wide D chunk
                for ch in range(4):
                    dw_t = mp_["dwp"].tile([128, 8, 512], F32R)
                    for hc in range(8):
                        nc.sync.dma_start(
                            dw_t[:, hc, :],
                            dw[hc * 128:(hc + 1) * 128,
                               ch * 512:(ch + 1) * 512])
                    for tb in range(TB):
                        psd = psM.tile([128, 512], F32, tag="psm")
                        for hc in range(8):
                            nc.tensor.matmul(
                                psd[:], actT[:, hc, tb * 128:(tb + 1) * 128],
                                dw_t[:, hc, :],
                                start=(hc == 0), stop=(hc == 7))
                        mb = obp.tile([128, 512], F32, tag="ob", name="mb")
                        nc.vector.tensor_copy(mb[:], psd[:])
                        nc.sync.dma_start(
                            mlp_chs[ch][b * T + tb * 128:
                                        b * T + (tb + 1) * 128, :], mb[:])
                    _rscatter(mlp_chs[ch][b * T:(b + 1) * T, :],
                              rs2_chs[ch][b * 128:(b + 1) * 128, :], fake_rs2)
                _ckpt(f"down{b}")
                _ckpt(f"rs2_{b}")

        # =================== final norm + residual ===================
        postffw_bc = bcast.tile([128, D], F32, tag="bc")
        _bcast_row(nc, postffw_bc[:], postffwmul)
        for b in range(B):
            rst = bwork.tile([128, D], F32, tag="bw")
            for ch in range(4):
                nc.sync.dma_start(rst[:, ch * 512:(ch + 1) * 512],
                                  rs2_chs[ch][b * 128:(b + 1) * 128, :])
            tmp = bwork.tile([128, D], F32, tag="bw")
            _rms(nc, pools, rst[:], tmp[:], postffw_bc[:], D, tmp[:])
            aol = bwork.tile([128, D], F32, tag="bw")
            _dma4(nc, aol[:], attn_out_d[b * 128:(b + 1) * 128, :])
            nc.vector.tensor_add(tmp[:], tmp[:], aol[:])
            _dma4(nc, out[b * 128:(b + 1) * 128, :], tmp[:])


# ---------------------------------------------------------------------------
# host side
# ---------------------------------------------------------------------------

_NC = None


def _get_nc():
    global _NC
    if _NC is None:
        _NC = _build_program()
    return _NC


def _host_prep(inputs):
    """Build the 8 per-core input maps from the full problem inputs."""
    x = np.ascontiguousarray(np.asarray(inputs["x"], dtype=np.float32))
    seg = np.asarray(inputs["segment_pos"], dtype=np.int32)
    am = np.asarray(inputs["attn_mask"])
    q_k = np.asarray(inputs["q_kernel"], dtype=np.float32)
    kv_k = np.asarray(inputs["kv_kernel"], dtype=np.float32)
    o_k = np.asarray(inputs["o_kernel"], dtype=np.float32)
    gate_w = np.asarray(inputs["gate_w"], dtype=np.float32)
    up_w = np.asarray(inputs["up_w"], dtype=np.float32)
    down_w = np.asarray(inputs["down_w"], dtype=np.float32)

    xf = x.reshape(B * T, D)
    premul = (1.0 + np.asarray(inputs["pre_attn_scale"], np.float32))
    postattn = (1.0 + np.asarray(inputs["post_attn_scale"], np.float32))
    preffw = (1.0 + np.asarray(inputs["pre_ffw_scale"], np.float32))
    postffw = (1.0 + np.asarray(inputs["post_ffw_scale"], np.float32))
    qmul = ((1.0 + np.asarray(inputs["q_norm_scale"], np.float32))
            * np.float32(H ** -0.5)).astype(np.float32)
    kmul = (1.0 + np.asarray(inputs["k_norm_scale"], np.float32))

    frac = (2.0 * np.arange(H // 2, dtype=np.float32) / H).astype(np.float32)
    ts = (ROPE_BASE ** frac).astype(np.float32)
    sinu = (seg[..., None].astype(np.float32) / ts).astype(np.float32)
    cosb = np.cos(sinu).reshape(B * T, 64).astype(np.float32)
    sinb = np.sin(sinu).reshape(B * T, 64).astype(np.float32)

    ti = np.arange(128)[:, None]
    si = np.arange(128)[None, :]
    maskb = np.stack([
        np.full((128, 128), KMASK, np.float32),
        np.where(ti >= si, 0.0, KMASK).astype(np.float32),
        np.zeros((128, 128), np.float32),
        np.where(ti < si, 0.0, KMASK).astype(np.float32),
    ]).astype(np.float32)

    # soft structural check of the actual mask against the canonical tiles
    tt = np.arange(T)
    sliding = (np.abs(tt[:, None] - tt[None, :]) <= WINDOW - 1)
    expected = am & sliding[None]
    ok = True
    for qb in range(min(2, TB)):
        for kb in range(qb + 1):
            blk = np.where(expected[0, qb * 128:(qb + 1) * 128,
                                    kb * 128:(kb + 1) * 128], 0.0, KMASK)
            if not np.array_equal(blk.astype(np.float32),
                                  maskb[_midx(qb, kb)]):
                ok = False
    if not ok:
        print("kernel.py WARNING: attn_mask does not match canonical "
              "causal+sliding structure; results may be wrong")

    iden = np.eye(128, dtype=np.float32)

    in_maps = []
    for c in range(NCORES):
        qw_c = q_k[2 * c:2 * c + 2].transpose(1, 0, 2).reshape(D, 256)
        kw_c = kv_k[0, c]
        vw_c = kv_k[1, c]
        wqkv_c = np.ascontiguousarray(
            np.concatenate([qw_c, kw_c, vw_c], axis=1), dtype=np.float32)
        ow_c = np.ascontiguousarray(
            o_k[2 * c:2 * c + 2].reshape(256, D), dtype=np.float32)
        gw_c = np.ascontiguousarray(gate_w[:, 1024 * c:1024 * (c + 1)])
        uw_c = np.ascontiguousarray(up_w[:, 1024 * c:1024 * (c + 1)])
        dw_c = np.ascontiguousarray(down_w[1024 * c:1024 * (c + 1), :])
        xsh_c = np.ascontiguousarray(np.concatenate(
            [xf[128 * c:128 * (c + 1)],
             xf[T + 128 * c: T + 128 * (c + 1)]], axis=0))
        in_maps.append({
            "x": xf, "xsh": xsh_c, "wqkv": wqkv_c, "ow": ow_c,
            "gw": gw_c, "uw": uw_c, "dw": dw_c,
            "cosb": cosb, "sinb": sinb, "maskb": maskb,
            "premul": premul, "postattnmul": postattn,
            "preffwmul": preffw, "postffwmul": postffw,
            "qmul": qmul, "kmul": kmul, "iden": iden,
        })
    return in_maps


def _assemble(results):
    out = np.empty((B, T, D), dtype=np.float32)
    for c in range(NCORES):
        r = results[c]["out"]
        out[0, 128 * c:128 * (c + 1)] = r[0:128]
        out[1, 128 * c:128 * (c + 1)] = r[128:256]
    return out


def kernel(**inputs) -> np.ndarray:
    from concourse import bass_utils
    nc = _get_nc()
    in_maps = _host_prep(inputs)
    r = bass_utils.run_bass_kernel_spmd(nc, in_maps,
                                        core_ids=list(range(NCORES)))
    return _assemble(r.results)
